# revision 19
# baseline (speedup 1.0000x reference)
"""Trainium2 Bass kernel for nn_CNN_9818295238933 (gnn_message_passing).

Data-parallel over batch across 8 cores (8 samples each). Per sample:
  conv1 (PE, bf16) -> h1 [32, F] -> REP matmul replicates h1 across 8
  partition groups as a bf16-pair-packed SBUF table [128, F] (partition
  (g, kp) holds the bf16 pair (h[2kp], h[2kp+1]) at face f).
  ap_gather (GPSIMD, SBUF-local) gathers the table with that sample's
  adjacency: groups 0-6 carry neighbour slot n for faces [0, FL); group 7
  carries the tail faces [FL, FG) of all 7 slots concatenated, so all 8
  Q7 cores work and each instruction processes FL/seg indices instead of
  FG. Gathered tiles feed the next conv directly as strided bf16 matmul
  rhs (contraction over (n, kp) partitions, even/odd k accumulated in
  PSUM); tail faces get per-n 16-partition matmuls from group 7's slice.
  Repeat for conv2 -> table2 -> gather -> conv3.

The ap_gather ucode is the hard bottleneck (~27.3 ns/index per Q7 core,
measured on idle HW; 48 segment-gathers x ~71.3 us = 3.42 ms). The
sample loop is software-pipelined so the Pool engine never waits:
gathers are emitted as [g2(s+1) segs][cc(s-1)][g3(s) segs]; convs are
emitted consumer-first (conv2(s+1), conv3(s) BEFORE conv1(s+2)) so PE
frees gather buffers promptly. All three tables (tab1 x2 live +
tab2 x1) share one tag-rotated 3-slot pool, which double-buffers tab1
at startup (kills the prologue stall) while keeping baseline SBUF use.

Head: idx(0)+conv1(0) emitted before all other constants. Tail: h3
bounces to DRAM per sample and a per-sample AllToAll redistributes
k-slices; fc1 runs as 3 16-col halves (iters 3/5/6) + sample-6 and
sample-7 8-col pieces; sample 7's AllToAll is split at face 2624 so
its first third (and fc1 piece7A) overlaps the final gather segment;
AllReduce is split in 3 column groups (2 fire during the loop).
BN+ReLU+fc2+BN+ReLU+fco replicated.

Self-contained: hardcodes all shapes; only imports the Trainium toolchain.
"""

import sys
from dataclasses import dataclass

if "/opt/trn_rl_repo" not in sys.path:
    sys.path.insert(0, "/opt/trn_rl_repo")

import numpy as np


@dataclass(frozen=True)
class Cfg:
    ncores: int = 8
    B: int = 64
    C: int = 12
    N: int = 7
    K: int = 32
    F: int = 9000
    FG: int = 9008          # compute/table extent (F padded to mult of 16)
    FL: int = 7888          # main faces per neighbour group (= FG * 7/8 pad16)
    H1: int = 100
    H2: int = 30
    NCLS: int = 2
    EPS: float = 1e-5
    CHUNK: int = 512        # PSUM f-chunk

    @property
    def BL(self):
        return self.B // self.ncores

    @property
    def CN(self):
        return self.C * self.N

    @property
    def KL(self):
        return self.K // self.ncores

    @property
    def KP(self):
        return self.K // 2

    @property
    def CHT(self):
        return self.N * self.KP  # 112 main channels

    @property
    def TL(self):
        return self.FG - self.FL  # 1120 tail faces

    @property
    def SEGS(self):
        # Segment starts must be multiples of 32 entries: the gather
        # ucode reads the wrapped idx list as u32 words, and a 2-byte
        # misaligned base corrupts words crossing 16-byte boundaries.
        if self.FL == 9008:  # tail disabled
            return [(0, 3008), (3008, 3008), (6016, 2992)]
        return [(0, 2624), (2624, 2624), (5248, 2640)]

    @property
    def WCOL(self):
        return self.FL // 16  # wrapped idx columns (493)

    @property
    def SPLIT(self):
        return self.SEGS[1][0]  # face split for sample-7's early AllToAll


CFG = Cfg()


def _chunks(f0, flen, step):
    out = []
    f = f0
    while f < f0 + flen:
        out.append((f, min(step, f0 + flen - f)))
        f += step
    return out


# ---------------------------------------------------------------------------
# Host-side input preparation
# ---------------------------------------------------------------------------

def prep_core_inputs(cfg: Cfg, x, adjacencies, W1, W2, W3, fc1_w, fc1_b, bn1_g,
                     bn1_b, fc2_w, fc2_b, bn2_g, bn2_b, fco_w, fco_b):
    import ml_dtypes
    bf16 = ml_dtypes.bfloat16

    B, C, N, K, F, FG, FL = (cfg.B, cfg.C, cfg.N, cfg.K, cfg.F, cfg.FG,
                             cfg.FL)
    BL, CN, KL, KP, TL = cfg.BL, cfg.CN, cfg.KL, cfg.KP, cfg.TL
    H1, H2, NCLS = cfg.H1, cfg.H2, cfg.NCLS

    x = np.asarray(x, dtype=np.float32)
    adj = np.asarray(adjacencies).astype(np.int64)[:, 0]  # [B, F, N]

    # x [B, C, F, N] -> xt [B, (c,n), FG] bf16, zero-padded along f.
    xt = np.zeros((B, CN, FG), dtype=bf16)
    xt[:, :, :F] = np.transpose(x, (0, 1, 3, 2)).reshape(B, CN, F).astype(bf16)

    # Gather index lists, one per 16-partition group:
    #   group n < 7: adj[b, f, n] for f in [0, FL)
    #   group 7:     adj[b, FL+u, n] at position n*TL+u (pad to FL with 0)
    # wrapped so entry i sits at [16g + i%16, i//16]. Segment boundaries
    # are multiples of 16 so column-slicing yields each segment's list.
    idx_pad = np.zeros((B, FG, N), dtype=np.int64)
    idx_pad[:, :F] = adj
    lists = np.zeros((B, 8, FL), dtype=np.int64)
    lists[:, :7, :] = np.transpose(idx_pad[:, :FL], (0, 2, 1))
    lists[:, 7, :N * TL] = np.transpose(
        idx_pad[:, FL:], (0, 2, 1)).reshape(B, N * TL)
    wrap = lists.reshape(B, 8, FL // 16, 16)
    idx16 = np.ascontiguousarray(
        np.transpose(wrap, (0, 1, 3, 2)).reshape(B, 128, FL // 16)
    ).astype(np.int16)

    w1f = np.transpose(np.asarray(W1, np.float32), (1, 2, 0)).reshape(CN, K)

    def eo(Wm):  # [K_out, K_in, N] -> even/odd lhsT [(n,kp), K_out] bf16
        Wm = np.asarray(Wm, np.float32)
        we = np.transpose(Wm[:, 0::2, :], (2, 1, 0)).reshape(N * KP, K)
        wo = np.transpose(Wm[:, 1::2, :], (2, 1, 0)).reshape(N * KP, K)
        return (np.ascontiguousarray(we).astype(bf16),
                np.ascontiguousarray(wo).astype(bf16))

    w2e, w2o = eo(W2)
    w3e, w3o = eo(W3)

    # Replication matrices over all 8 groups: repe[q, (g,kp)] = (q == 2*kp)
    q = np.arange(K)[:, None]
    p = np.arange(128)[None, :]
    repe = (q == 2 * (p % KP)).astype(bf16)
    repo = (q == 2 * (p % KP) + 1).astype(bf16)

    # fc1 weights: [H1, K*F] -> [K, FG, H1] zero-padded, per-core k-slice.
    fc1 = np.asarray(fc1_w, np.float32).reshape(H1, K, F)
    fc1t = np.zeros((K, FG, H1), dtype=bf16)
    fc1t[:, :F] = np.transpose(fc1, (1, 2, 0)).astype(bf16)

    fc2wt = np.ascontiguousarray(np.asarray(fc2_w, np.float32).T)  # [H1, H2]
    fcowt = np.ascontiguousarray(np.asarray(fco_w, np.float32).T)  # [H2, NCLS]

    def col(v, n):
        return np.asarray(v, np.float32).reshape(n, 1)

    shared = dict(
        w1=w1f.astype(bf16), w2e=w2e, w2o=w2o, w3e=w3e, w3o=w3o,
        repe=repe, repo=repo,
        fc1b=col(fc1_b, H1), bn1g=col(bn1_g, H1), bn1b=col(bn1_b, H1),
        fc2wt=fc2wt, fc2b=col(fc2_b, H2), bn2g=col(bn2_g, H2),
        bn2b=col(bn2_b, H2), fcowt=fcowt, fcob=col(fco_b, NCLS),
    )

    in_maps = []
    for c in range(cfg.ncores):
        bsl = slice(c * BL, (c + 1) * BL)
        fc1wt_c = np.ascontiguousarray(
            fc1t[c * KL:(c + 1) * KL].reshape(KL * FG, H1))
        m = dict(shared)
        m.update(
            xt=np.ascontiguousarray(xt[bsl]),
            idx16=np.ascontiguousarray(idx16[bsl]),
            fc1wt=fc1wt_c,
        )
        in_maps.append(m)
    return in_maps


def postprocess(out_dev: np.ndarray, cfg: Cfg = CFG) -> np.ndarray:
    """Device out columns are (sample-within-core, core) ordered; return
    [B, NCLS] in global sample order (core-major)."""
    o = np.asarray(out_dev, np.float32).reshape(cfg.NCLS, cfg.BL, cfg.ncores)
    return np.ascontiguousarray(o.transpose(2, 1, 0).reshape(cfg.B, cfg.NCLS))


# ---------------------------------------------------------------------------
# Device program
# ---------------------------------------------------------------------------

def build_program(cfg: Cfg):
    import concourse.bass as bass  # noqa: F401
    import concourse.bacc as bacc
    import concourse.mybir as mybir
    import concourse.tile as tile
    from concourse.masks import make_identity

    dt = mybir.dt.float32
    bf = mybir.dt.bfloat16
    u32 = mybir.dt.uint32
    i16 = mybir.dt.int16
    B, C, N, K, FG, FL = cfg.B, cfg.C, cfg.N, cfg.K, cfg.FG, cfg.FL
    BL, CN, KL, KP, CHT, TL = (cfg.BL, cfg.CN, cfg.KL, cfg.KP, cfg.CHT,
                               cfg.TL)
    H1, H2, NCLS = cfg.H1, cfg.H2, cfg.NCLS
    CHUNK, SEGS, WCOL = cfg.CHUNK, cfg.SEGS, cfg.WCOL
    SPLIT = cfg.SPLIT
    NCORES = cfg.ncores
    SEGMAX = max(w for _, w in SEGS)
    BLK = 1024
    rg = [list(range(NCORES))]

    nc = bacc.Bacc("TRN2", target_bir_lowering=False, debug=False,
                   num_devices=NCORES, num_swdge_queues=4)

    xt = nc.dram_tensor("xt", [BL, CN, FG], bf, kind="ExternalInput")
    idx16 = nc.dram_tensor("idx16", [BL, 128, WCOL], i16,
                           kind="ExternalInput")
    w1 = nc.dram_tensor("w1", [CN, K], bf, kind="ExternalInput")
    w2e = nc.dram_tensor("w2e", [CHT, K], bf, kind="ExternalInput")
    w2o = nc.dram_tensor("w2o", [CHT, K], bf, kind="ExternalInput")
    w3e = nc.dram_tensor("w3e", [CHT, K], bf, kind="ExternalInput")
    w3o = nc.dram_tensor("w3o", [CHT, K], bf, kind="ExternalInput")
    repe = nc.dram_tensor("repe", [K, 128], bf, kind="ExternalInput")
    repo = nc.dram_tensor("repo", [K, 128], bf, kind="ExternalInput")
    fc1wt = nc.dram_tensor("fc1wt", [KL * FG, H1], bf, kind="ExternalInput")
    fc1b = nc.dram_tensor("fc1b", [H1, 1], dt, kind="ExternalInput")
    bn1g = nc.dram_tensor("bn1g", [H1, 1], dt, kind="ExternalInput")
    bn1b = nc.dram_tensor("bn1b", [H1, 1], dt, kind="ExternalInput")
    fc2wt = nc.dram_tensor("fc2wt", [H1, H2], dt, kind="ExternalInput")
    fc2b = nc.dram_tensor("fc2b", [H2, 1], dt, kind="ExternalInput")
    bn2g = nc.dram_tensor("bn2g", [H2, 1], dt, kind="ExternalInput")
    bn2b = nc.dram_tensor("bn2b", [H2, 1], dt, kind="ExternalInput")
    fcowt = nc.dram_tensor("fcowt", [H2, NCLS], dt, kind="ExternalInput")
    fcob = nc.dram_tensor("fcob", [NCLS, 1], dt, kind="ExternalInput")
    out = nc.dram_tensor("out", [NCLS, B], dt, kind="ExternalOutput")

    def tail_pieces():
        """(n, seg_idx, seg_local_start, width, tail_local_start) pieces
        covering each neighbour's [n*TL, (n+1)*TL) slice of group-7's
        entry list, split at gather-segment boundaries."""
        out_runs = []
        for n in range(N):
            e0, e1 = n * TL, (n + 1) * TL
            for si, (s0, slen) in enumerate(SEGS):
                lo = max(e0, s0)
                hi = min(e1, s0 + slen)
                if lo < hi:
                    out_runs.append((n, si, lo - s0, hi - lo, lo - e0))
        return out_runs

    with tile.TileContext(nc) as tc:
        with (
            tc.tile_pool(name="consts", bufs=1) as consts,
            tc.tile_pool(name="xcp", bufs=2) as xcp,
            tc.tile_pool(name="idxp", bufs=3) as idxp,
            tc.tile_pool(name="tabs", bufs=3) as tabsp,
            tc.tile_pool(name="gop", bufs=4) as gop,
            tc.tile_pool(name="tbp", bufs=1) as tbp,
            tc.tile_pool(name="hp", bufs=1) as hp,
            tc.tile_pool(name="hst", bufs=2) as hstp,
            tc.tile_pool(name="work", bufs=2) as work,
            tc.tile_pool(name="xbp", bufs=2) as xbp,
            tc.tile_pool(name="dram", bufs=1, space="DRAM") as dram,
        ):
            # ---- constants needed by conv1(0) go first; the rest are
            # emitted after the first gathers so they don't delay the head.
            w1_t = consts.tile([CN, K], bf)
            nc.sync.dma_start(w1_t[:], w1[:])
            repe_t = consts.tile([K, 128], bf)
            nc.sync.dma_start(repe_t[:], repe[:])
            repo_t = consts.tile([K, 128], bf)
            nc.sync.dma_start(repo_t[:], repo[:])

            bounce = dram.tile([BL, NCORES, KL, FG], bf)
            recv = dram.tile([BL, NCORES, KL, FG], bf)
            # sample-7 AllToAll face-range splits (seg0 / seg1 / seg2+tail)
            S7R = [(0, SEGS[1][0]), (SEGS[1][0], SEGS[2][0]),
                   (SEGS[2][0], FG)]
            bounce7 = [dram.tile([NCORES, KL, hi - lo], bf,
                                 name=f"bounce7_{i}")
                       for i, (lo, hi) in enumerate(S7R)]
            recv7 = [dram.tile([NCORES, KL, hi - lo], bf,
                               name=f"recv7_{i}")
                     for i, (lo, hi) in enumerate(S7R)]
            y1snd1 = dram.tile([H1, 32], dt)
            y1rcv1 = dram.tile([H1, 32], dt)
            y1snd2 = dram.tile([H1, 32], dt)
            y1rcv2 = dram.tile([H1, 32], dt)

            with (
                tc.tile_pool(name="cpsum", bufs=2, space="PSUM") as cpsum,
                tc.tile_pool(name="rpsum", bufs=2, space="PSUM") as rpsum,
                tc.tile_pool(name="fpsum", bufs=1, space="PSUM") as fpsum,
            ):
                def build_table(tab, hs, f0, w):
                    """REP-matmul an h chunk [K, w] into the packed table."""
                    tb = tab[:].bitcast(bf).rearrange(
                        "p (f two) -> p f two", two=2)
                    pse = rpsum.tile([128, CHUNK], dt, tag="rp")
                    nc.tensor.matmul(out=pse[:, :w], lhsT=repe_t[:],
                                     rhs=hs[:, :w], start=True, stop=True)
                    nc.vector.tensor_copy(tb[:, f0:f0 + w, 0], pse[:, :w])
                    pso = rpsum.tile([128, CHUNK], dt, tag="rp")
                    nc.tensor.matmul(out=pso[:, :w], lhsT=repo_t[:],
                                     rhs=hs[:, :w], start=True, stop=True)
                    nc.scalar.copy(tb[:, f0:f0 + w, 1], pso[:, :w])

                st = {}  # per-sample tile state

                def load_idx(s):
                    idx_t = idxp.tile([128, WCOL], i16, tag="it")
                    nc.sync.dma_start(idx_t[:], idx16[s])
                    st[s] = dict(idx=idx_t)

                XBLK = 2252  # x staged in 4 big DMAs instead of 18 small

                def conv1_full(s, tab1):
                    st[s]["tab1"] = tab1
                    for xb0, xbw in _chunks(0, FG, XBLK):
                        xc = xbp.tile([CN, XBLK], bf, tag="xb")
                        nc.sync.dma_start(xc[:, :xbw],
                                          xt[s, :, xb0:xb0 + xbw])
                        for f0, w in _chunks(xb0, xbw, CHUNK):
                            lo = f0 - xb0
                            ps = cpsum.tile([K, CHUNK], dt, tag="cp")
                            nc.tensor.matmul(out=ps[:, :w], lhsT=w1_t[:],
                                             rhs=xc[:, lo:lo + w],
                                             start=True, stop=True)
                            hs = hstp.tile([K, CHUNK], bf, tag="hst")
                            nc.vector.tensor_copy(hs[:, :w], ps[:, :w])
                            build_table(tab1, hs, f0, w)

                def gather_seg(s, tab_key, seg):
                    s0, slen = SEGS[seg]
                    go = gop.tile([128, SEGMAX], u32, tag="go")
                    nc.gpsimd.ap_gather(
                        out_ap=go[:, :slen], in_ap=st[s][tab_key][:],
                        idxs_ap=st[s]["idx"][:, s0 // 16:(s0 + slen) // 16],
                        channels=128, num_elems=FG, d=1, num_idxs=slen)
                    return go

                def stage_tail(gos):
                    """SBUF->SBUF DMA group-7's gathered entries into the
                    main (n, kp) partition layout: tailbuf[16n+kp, u] =
                    go[112+kp, n*TL+u]. Returns the [CHT, TL] u32 tile."""
                    tb = tbp.tile([CHT, TL], u32, tag="tb")
                    for n, si, lo, rw, u0 in tail_pieces():
                        nc.sync.dma_start(
                            tb[16 * n:16 * n + 16, u0:u0 + rw],
                            gos[si][112:128, lo:lo + rw])
                    return tb

                def conv_segs(gos, we_t, wo_t, sink, seg_ids):
                    """Conv faces covered by the given gather segments."""
                    for si in seg_ids:
                        s0, slen = SEGS[si]
                        gb = gos[si][:, :slen].bitcast(bf).rearrange(
                            "p (f two) -> p f two", two=2)
                        for f0, w in _chunks(s0, slen, CHUNK):
                            lo = f0 - s0
                            ps = cpsum.tile([K, CHUNK], dt, tag="cp")
                            nc.tensor.matmul(out=ps[:, :w],
                                             lhsT=we_t[:],
                                             rhs=gb[0:CHT, lo:lo + w, 0],
                                             start=True, stop=False)
                            nc.tensor.matmul(out=ps[:, :w],
                                             lhsT=wo_t[:],
                                             rhs=gb[0:CHT, lo:lo + w, 1],
                                             start=False, stop=True)
                            sink(f0, w, ps)

                def conv_tail(gos, we_t, wo_t, sink):
                    """Conv the tail faces [FL, FG) via the restaged
                    group-7 entries."""
                    if FL >= FG:
                        return
                    tb = stage_tail(gos)
                    tbb = tb[:].bitcast(bf).rearrange(
                        "p (f two) -> p f two", two=2)
                    for f0, w in _chunks(FL, FG - FL, CHUNK):
                        lo = f0 - FL
                        ps = cpsum.tile([K, CHUNK], dt, tag="cp")
                        nc.tensor.matmul(out=ps[:, :w], lhsT=we_t[:],
                                         rhs=tbb[:, lo:lo + w, 0],
                                         start=True, stop=False)
                        nc.tensor.matmul(out=ps[:, :w], lhsT=wo_t[:],
                                         rhs=tbb[:, lo:lo + w, 1],
                                         start=False, stop=True)
                        sink(f0, w, ps)

                def conv2_full(s, gos, tab2):
                    st[s]["tab2"] = tab2

                    def sink(f0, w, ps):
                        hs = hstp.tile([K, CHUNK], bf, tag="hst")
                        nc.vector.tensor_copy(hs[:, :w], ps[:, :w])
                        build_table(tab2, hs, f0, w)
                    conv_segs(gos, w2e_t, w2o_t, sink, [0, 1, 2])
                    conv_tail(gos, w2e_t, w2o_t, sink)

                def conv3_sink(s):
                    hs3 = hp.tile([K, FG], bf, tag="h3", name=f"h3_{s}")

                    def sink(f0, w, ps):
                        nc.vector.tensor_copy(hs3[:, f0:f0 + w], ps[:, :w])
                    return hs3, sink

                def conv3_full(s, gos):
                    hs3, sink = conv3_sink(s)
                    conv_segs(gos, w3e_t, w3o_t, sink, [0, 1, 2])
                    conv_tail(gos, w3e_t, w3o_t, sink)
                    # single bounce write per sample: the AllToAll's input
                    # must have one writer (chunked writers race the
                    # collective on HW).
                    nc.sync.dma_start(bounce[s], hs3[:])

                def cc_sample(s):
                    nc.gpsimd.collective_compute(
                        "AllToAll", mybir.AluOpType.bypass,
                        replica_groups=rg,
                        ins=[bounce[s].opt()], outs=[recv[s].opt()])

                # ---- fc1: y1ps[:, cols] += fc1wt.T @ transposed recv rows,
                # accumulated per column group over KL x 71 PSUM chunks.
                y1ps = fpsum.tile([H1, B], dt, tag="y1")
                fc1_state = {}

                def fc1_part(grp, c0, ncols, bblocks, load_rows, total_nst):
                    stt = fc1_state.setdefault(grp, dict(stp=0))
                    for kl in range(KL):
                        for b0, bw in bblocks:
                            lt_in = work.tile([ncols, BLK], bf,
                                              tag=f"ltin{ncols}")
                            load_rows(lt_in, kl, b0, bw)
                            r0 = kl * FG + b0
                            nfull = bw // 128
                            wt = work.tile([128, (BLK // 128) * H1], bf,
                                           tag="fw")
                            if nfull:
                                nc.sync.dma_start(
                                    wt[:, :nfull * H1].rearrange(
                                        "p (c h) -> p c h", h=H1),
                                    fc1wt[r0:r0 + nfull * 128, :].rearrange(
                                        "(c p) h -> p c h", p=128))
                            for ci, (s0c, wc) in enumerate(
                                    _chunks(0, bw, 128)):
                                pst = rpsum.tile([128, 16], bf, tag="tT")
                                nc.tensor.transpose(
                                    pst[:wc, :ncols],
                                    lt_in[:, s0c:s0c + wc],
                                    identB[:ncols, :ncols])
                                ltt = work.tile([128, ncols], bf,
                                                tag=f"ltt{ncols}")
                                nc.vector.tensor_copy(ltt[:wc, :],
                                                      pst[:wc, :ncols])
                                if wc == 128:
                                    lhsT = wt[:, ci * H1:(ci + 1) * H1]
                                else:
                                    wtp = work.tile([128, H1], bf, tag="fwp")
                                    nc.sync.dma_start(
                                        wtp[:wc, :],
                                        fc1wt[r0 + s0c:r0 + s0c + wc, :])
                                    lhsT = wtp[:wc, :]
                                nc.tensor.matmul(
                                    out=y1ps[:, c0:c0 + ncols],
                                    lhsT=lhsT, rhs=ltt[:wc, :],
                                    start=(stt["stp"] == 0),
                                    stop=(stt["stp"] == total_nst - 1))
                                stt["stp"] += 1

                FULL_BLOCKS = _chunks(0, FG, BLK)
                NST_FULL = KL * sum(len(_chunks(0, bw, 128))
                                    for _, bw in FULL_BLOCKS)

                def fc1_half(h):
                    def load_rows(t, kl, b0, bw):
                        nc.sync.dma_start(
                            t[:, :bw], recv[2 * h:2 * h + 2, :, kl,
                                            b0:b0 + bw])
                    fc1_part(f"h{h}", 16 * h, 16, FULL_BLOCKS, load_rows,
                             NST_FULL)

                BLOCKS_7 = [_chunks(lo, hi - lo, BLK) for lo, hi in S7R]
                NST_P7 = KL * sum(len(_chunks(0, bw, 128))
                                  for blocks in BLOCKS_7
                                  for _, bw in blocks)

                def fc1_half3(part):
                    """cols 48:64 = samples 6,7; face-range `part` so each
                    part unblocks as soon as its sample-7 AllToAll lands."""
                    src, off = recv7[part], S7R[part][0]

                    def load_rows(t, kl, b0, bw):
                        nc.sync.dma_start(
                            t[0:8, :bw], recv[6:7, :, kl, b0:b0 + bw])
                        nc.sync.dma_start(
                            t[8:16, :bw],
                            src[:, kl, b0 - off:b0 - off + bw])
                    fc1_part("h3", 48, 16, BLOCKS_7[part], load_rows, NST_P7)

                def new_tab(kind, s):
                    return tabsp.tile([128, FG], u32, tag="tab",
                                      name=f"tab{kind}_{s}")

                def a2a(src, dst):
                    nc.gpsimd.collective_compute(
                        "AllToAll", mybir.AluOpType.bypass,
                        replica_groups=rg,
                        ins=[src[:].opt()], outs=[dst[:].opt()])

                def allreduce(src, dst):
                    nc.gpsimd.collective_compute(
                        "AllReduce", mybir.AluOpType.add, replica_groups=rg,
                        ins=[src[:].opt()], outs=[dst[:].opt()])

                # ---- prologue: sample-0 table + first gathers before all
                # other constants, so the head is just idx0+conv1(0).
                load_idx(0)
                # keep the (otherwise unused) xcp pool's footprint so the
                # SBUF layout of the pools behind it doesn't shift — the
                # ap_gather ucode rate is sensitive to table placement.
                xpad0 = xcp.tile([CN, CHUNK], bf, tag="xc")
                nc.vector.memset(xpad0[:, :8], 0.0)
                xpad1 = xcp.tile([CN, CHUNK], bf, tag="xc")
                nc.vector.memset(xpad1[:, :8], 0.0)
                # dummy 4-index gather: triggers the ~70us Q7 gather-library
                # IRAM load now, overlapped with conv1(0), instead of
                # stalling the first real gather on it.
                nc.gpsimd.ap_gather(
                    out_ap=xpad1[:].bitcast(u32)[0:16, 0:4],
                    in_ap=xpad0[:].bitcast(u32)[0:16, 0:4],
                    idxs_ap=xpad0[:].bitcast(i16)[0:16, 8:9],
                    channels=16, num_elems=4, d=1, num_idxs=4)
                conv1_full(0, new_tab(1, 0))
                gos0 = [gather_seg(0, "tab1", i) for i in range(3)]
                load_idx(1)
                conv1_full(1, new_tab(1, 1))

                w2e_t = consts.tile([CHT, K], bf)
                nc.sync.dma_start(w2e_t[:], w2e[:])
                w2o_t = consts.tile([CHT, K], bf)
                nc.sync.dma_start(w2o_t[:], w2o[:])
                w3e_t = consts.tile([CHT, K], bf)
                nc.sync.dma_start(w3e_t[:], w3e[:])
                w3o_t = consts.tile([CHT, K], bf)
                nc.sync.dma_start(w3o_t[:], w3o[:])
                identB = consts.tile([B, B], bf)
                make_identity(nc, identB)
                zcol = consts.tile([128, 1], dt)
                nc.vector.memset(zcol[:], 0.0)

                conv2_full(0, gos0, new_tab(2, 0))

                # ---- software-pipelined sample loop ----
                for s in range(BL):
                    nxt = s + 1 < BL
                    # table slot rotation in conv1-then-conv2 order (the
                    # 3-slot cycle then always lands writers on slots whose
                    # readers finished an iteration ago)
                    t1n = new_tab(1, s + 2) if s + 2 < BL else None
                    t2n = new_tab(2, s + 1) if nxt else None
                    if nxt:
                        gos2 = [gather_seg(s + 1, "tab1", i)
                                for i in range(3)]
                    if s >= 1:
                        cc_sample(s - 1)
                    if s < BL - 1:
                        gos3 = [gather_seg(s, "tab2", i) for i in range(3)]
                        if s + 2 < BL:
                            load_idx(s + 2)
                            conv1_full(s + 2, t1n)
                        if nxt:
                            conv2_full(s + 1, gos2, t2n)
                        conv3_full(s, gos3)
                        if s == 3:
                            fc1_half(0)
                        elif s == 5:
                            fc1_half(1)
                        elif s == 6:
                            # AllReduce of halves 0,1 (cols 0:32): fired
                            # inside the loop, off the tail critical path
                            y1l1 = work.tile([H1, 32], dt, tag="y1l1")
                            nc.vector.tensor_copy(y1l1[:], y1ps[:, 0:32])
                            nc.sync.dma_start(y1snd1[:], y1l1[:])
                            allreduce(y1snd1, y1rcv1)
                    else:
                        # ---- last sample: half2 fills iter-7's PE slack;
                        # the AllToAll is split in 3 face ranges so the
                        # exchange and fc1 half3 overlap the final gathers.
                        g3a = gather_seg(s, "tab2", 0)
                        g3b = gather_seg(s, "tab2", 1)
                        hs3, sink = conv3_sink(s)
                        conv_segs([g3a], w3e_t, w3o_t, sink, [0])
                        nc.sync.dma_start(bounce7[0][:],
                                          hs3[:, S7R[0][0]:S7R[0][1]])
                        fc1_half(2)
                        g3c = gather_seg(s, "tab2", 2)
                        gos3 = [g3a, g3b, g3c]
                        # triggers sit after all gathers in the Pool queue:
                        # the sequencer runs ahead of the gather engine, so
                        # each fires as soon as its bounce DMA lands.
                        a2a(bounce7[0], recv7[0])
                        fc1_half3(0)
                        conv_segs(gos3, w3e_t, w3o_t, sink, [1])
                        nc.sync.dma_start(bounce7[1][:],
                                          hs3[:, S7R[1][0]:S7R[1][1]])
                        a2a(bounce7[1], recv7[1])
                        fc1_half3(1)
                        conv_segs(gos3, w3e_t, w3o_t, sink, [2])
                        conv_tail(gos3, w3e_t, w3o_t, sink)
                        nc.sync.dma_start(bounce7[2][:],
                                          hs3[:, S7R[2][0]:S7R[2][1]])
                        a2a(bounce7[2], recv7[2])
                        fc1_half3(2)
                        y1l2 = work.tile([H1, 32], dt, tag="y1l2")
                        nc.vector.tensor_copy(y1l2[:], y1ps[:, 32:64])
                        nc.sync.dma_start(y1snd2[:], y1l2[:])
                        allreduce(y1snd2, y1rcv2)
                    st.pop(s)

                # ---- head (replicated) ----
                def bn_relu(y, h, g_ap, b_ap, relu=True):
                    """In-place batchnorm(+relu) on SBUF tile y [h, B]."""
                    mean = work.tile([h, 1], dt, tag=f"bn_m{h}")
                    nc.vector.reduce_sum(mean[:], y[:],
                                         axis=mybir.AxisListType.X)
                    nc.vector.tensor_scalar_mul(mean[:], mean[:], 1.0 / B)
                    sq = work.tile([h, B], dt, tag=f"bn_sq{h}")
                    nc.vector.tensor_tensor(out=sq[:], in0=y[:], in1=y[:],
                                            op=mybir.AluOpType.mult)
                    var = work.tile([h, 1], dt, tag=f"bn_v{h}")
                    nc.vector.reduce_sum(var[:], sq[:],
                                         axis=mybir.AxisListType.X)
                    nc.vector.tensor_scalar_mul(var[:], var[:], 1.0 / B)
                    m2 = work.tile([h, 1], dt, tag=f"bn_m2{h}")
                    nc.vector.tensor_tensor(out=m2[:], in0=mean[:],
                                            in1=mean[:],
                                            op=mybir.AluOpType.mult)
                    nc.vector.tensor_tensor(out=var[:], in0=var[:], in1=m2[:],
                                            op=mybir.AluOpType.subtract)
                    nc.vector.tensor_scalar_add(var[:], var[:], cfg.EPS)
                    std = work.tile([h, 1], dt, tag=f"bn_s{h}")
                    nc.scalar.activation(std[:], var[:],
                                         mybir.ActivationFunctionType.Sqrt,
                                         bias=zcol[:h, :1])
                    rstd = work.tile([h, 1], dt, tag=f"bn_r{h}")
                    nc.vector.reciprocal(rstd[:], std[:])
                    gl = work.tile([h, 1], dt, tag=f"bn_g{h}")
                    nc.sync.dma_start(gl[:], g_ap[:])
                    bl = work.tile([h, 1], dt, tag=f"bn_b{h}")
                    nc.sync.dma_start(bl[:], b_ap[:])
                    scale = work.tile([h, 1], dt, tag=f"bn_sc{h}")
                    nc.vector.tensor_tensor(out=scale[:], in0=rstd[:],
                                            in1=gl[:],
                                            op=mybir.AluOpType.mult)
                    shift = work.tile([h, 1], dt, tag=f"bn_sh{h}")
                    nc.vector.tensor_tensor(out=shift[:], in0=mean[:],
                                            in1=scale[:],
                                            op=mybir.AluOpType.mult)
                    nc.vector.tensor_tensor(out=shift[:], in0=bl[:],
                                            in1=shift[:],
                                            op=mybir.AluOpType.subtract)
                    nc.vector.tensor_scalar(
                        out=y[:], in0=y[:], scalar1=scale[:], scalar2=shift[:],
                        op0=mybir.AluOpType.mult, op1=mybir.AluOpType.add)
                    if relu:
                        nc.scalar.activation(y[:], y[:],
                                             mybir.ActivationFunctionType.Relu,
                                             bias=zcol[:h, :1])

                y1 = work.tile([H1, B], dt, tag="y1h")
                nc.sync.dma_start(y1[:, 0:32], y1rcv1[:])
                nc.sync.dma_start(y1[:, 32:64], y1rcv2[:])
                f1b = work.tile([H1, 1], dt, tag="f1b")
                nc.sync.dma_start(f1b[:], fc1b[:])
                nc.vector.tensor_scalar_add(y1[:], y1[:], f1b[:])
                bn_relu(y1, H1, bn1g, bn1b)

                w2f = work.tile([H1, H2], dt, tag="w2f")
                nc.sync.dma_start(w2f[:], fc2wt[:])
                ps2 = cpsum.tile([K, CHUNK], dt, tag="cp")
                nc.tensor.matmul(out=ps2[0:H2, 0:B], lhsT=w2f[:], rhs=y1[:],
                                 start=True, stop=True)
                y2 = work.tile([H2, B], dt, tag="y2h")
                nc.vector.tensor_copy(y2[:], ps2[0:H2, 0:B])
                f2b = work.tile([H2, 1], dt, tag="f2b")
                nc.sync.dma_start(f2b[:], fc2b[:])
                nc.vector.tensor_scalar_add(y2[:], y2[:], f2b[:])
                bn_relu(y2, H2, bn2g, bn2b)

                wof = work.tile([H2, NCLS], dt, tag="wof")
                nc.sync.dma_start(wof[:], fcowt[:])
                pso = cpsum.tile([K, CHUNK], dt, tag="cp")
                nc.tensor.matmul(out=pso[0:NCLS, 0:B], lhsT=wof[:], rhs=y2[:],
                                 start=True, stop=True)
                yo = work.tile([NCLS, B], dt, tag="yo")
                nc.vector.tensor_copy(yo[:], pso[0:NCLS, 0:B])
                fob = work.tile([NCLS, 1], dt, tag="fob")
                nc.sync.dma_start(fob[:], fcob[:])
                nc.vector.tensor_scalar_add(yo[:], yo[:], fob[:])
                nc.sync.dma_start(out[:], yo[:])

    nc.compile()
    return nc


_CACHE: dict = {}


def _get_program(cfg: Cfg):
    key = cfg
    if key not in _CACHE:
        _CACHE[key] = build_program(cfg)
    return _CACHE[key]


def kernel(**inputs) -> np.ndarray:
    from concourse import bass_utils

    cfg = CFG
    nc = _get_program(cfg)
    in_maps = prep_core_inputs(cfg, **inputs)
    res = bass_utils.run_bass_kernel_spmd(
        nc, in_maps, core_ids=list(range(cfg.ncores)))
    return postprocess(res.results[0]["out"], cfg)


# revision 20
# speedup vs baseline: 1.1953x; 1.1953x over previous
"""Trainium2 Bass kernel for nn_CNN_9818295238933 (gnn_message_passing).

Data-parallel over batch across 8 cores (8 samples each). Per sample:
  conv1 (PE, bf16) -> h1 [32, F] -> REP matmul replicates h1 across 8
  partition groups as a bf16-pair-packed SBUF table [128, F] (partition
  (g, kp) holds the bf16 pair (h[2kp], h[2kp+1]) at face f).
  ap_gather (GPSIMD, SBUF-local) gathers the table with that sample's
  adjacency: groups 0-6 carry neighbour slot n for faces [0, FL); group 7
  carries the tail faces [FL, FG) of all 7 slots concatenated, so all 8
  Q7 cores work and each instruction processes FL/seg indices instead of
  FG. Gathered tiles feed the next conv directly as strided bf16 matmul
  rhs (contraction over (n, kp) partitions, even/odd k accumulated in
  PSUM); tail faces get per-n 16-partition matmuls from group 7's slice.
  Repeat for conv2 -> table2 -> gather -> conv3.

The ap_gather ucode is the hard bottleneck (~27.3 ns/index per Q7 core,
measured on idle HW; 48 segment-gathers x ~71.3 us = 3.42 ms). The
sample loop is software-pipelined so the Pool engine never waits:
gathers are emitted as [g2(s+1) segs][cc(s-1)][g3(s) segs]; convs are
emitted consumer-first (conv2(s+1), conv3(s) BEFORE conv1(s+2)) so PE
frees gather buffers promptly. All three tables (tab1 x2 live +
tab2 x1) share one tag-rotated 3-slot pool, which double-buffers tab1
at startup (kills the prologue stall) while keeping baseline SBUF use.

Head: idx(0)+conv1(0) emitted before all other constants. Tail: h3
bounces to DRAM per sample and a per-sample AllToAll redistributes
k-slices; fc1 runs as 3 16-col halves (iters 3/5/6) + sample-6 and
sample-7 8-col pieces; sample 7's AllToAll is split at face 2624 so
its first third (and fc1 piece7A) overlaps the final gather segment;
AllReduce is split in 3 column groups (2 fire during the loop).
BN+ReLU+fc2+BN+ReLU+fco replicated.

Self-contained: hardcodes all shapes; only imports the Trainium toolchain.
"""

import sys
from dataclasses import dataclass

if "/opt/trn_rl_repo" not in sys.path:
    sys.path.insert(0, "/opt/trn_rl_repo")

import numpy as np


@dataclass(frozen=True)
class Cfg:
    ncores: int = 8
    B: int = 64
    C: int = 12
    N: int = 7
    K: int = 32
    F: int = 9000
    FG: int = 9008          # compute/table extent (F padded to mult of 16)
    FL: int = 7888          # main faces per neighbour group (= FG * 7/8 pad16)
    H1: int = 100
    H2: int = 30
    NCLS: int = 2
    EPS: float = 1e-5
    CHUNK: int = 512        # PSUM f-chunk

    @property
    def BL(self):
        return self.B // self.ncores

    @property
    def CN(self):
        return self.C * self.N

    @property
    def KL(self):
        return self.K // self.ncores

    @property
    def KP(self):
        return self.K // 2

    @property
    def CHT(self):
        return self.N * self.KP  # 112 main channels

    @property
    def TL(self):
        return self.FG - self.FL  # 1120 tail faces

    @property
    def SEGS(self):
        # Segment starts must be multiples of 32 entries: the gather
        # ucode reads the wrapped idx list as u32 words, and a 2-byte
        # misaligned base corrupts words crossing 16-byte boundaries.
        if self.FL == 9008:  # tail disabled
            return [(0, 3008), (3008, 3008), (6016, 2992)]
        return [(0, 2624), (2624, 2624), (5248, 2640)]

    @property
    def WCOL(self):
        return self.FL // 16  # wrapped idx columns (493)

    @property
    def SPLIT(self):
        return self.SEGS[1][0]  # face split for sample-7's early AllToAll


CFG = Cfg()


def _chunks(f0, flen, step):
    out = []
    f = f0
    while f < f0 + flen:
        out.append((f, min(step, f0 + flen - f)))
        f += step
    return out


# ---------------------------------------------------------------------------
# Host-side input preparation
# ---------------------------------------------------------------------------

def prep_core_inputs(cfg: Cfg, x, adjacencies, W1, W2, W3, fc1_w, fc1_b, bn1_g,
                     bn1_b, fc2_w, fc2_b, bn2_g, bn2_b, fco_w, fco_b):
    import ml_dtypes
    bf16 = ml_dtypes.bfloat16

    B, C, N, K, F, FG, FL = (cfg.B, cfg.C, cfg.N, cfg.K, cfg.F, cfg.FG,
                             cfg.FL)
    BL, CN, KL, KP, TL = cfg.BL, cfg.CN, cfg.KL, cfg.KP, cfg.TL
    H1, H2, NCLS = cfg.H1, cfg.H2, cfg.NCLS

    x = np.asarray(x, dtype=np.float32)
    adj = np.asarray(adjacencies).astype(np.int64)[:, 0]  # [B, F, N]

    # x [B, C, F, N] -> xt [B, (c,n), FG] bf16, zero-padded along f.
    xt = np.zeros((B, CN, FG), dtype=bf16)
    xt[:, :, :F] = np.transpose(x, (0, 1, 3, 2)).reshape(B, CN, F).astype(bf16)

    # Gather index lists, one per 16-partition group:
    #   group n < 7: adj[b, f, n] for f in [0, FL)
    #   group 7:     adj[b, FL+u, n] at position n*TL+u (pad to FL with 0)
    # wrapped so entry i sits at [16g + i%16, i//16]. Segment boundaries
    # are multiples of 16 so column-slicing yields each segment's list.
    idx_pad = np.zeros((B, FG, N), dtype=np.int64)
    idx_pad[:, :F] = adj
    lists = np.zeros((B, 8, FL), dtype=np.int64)
    lists[:, :7, :] = np.transpose(idx_pad[:, :FL], (0, 2, 1))
    lists[:, 7, :N * TL] = np.transpose(
        idx_pad[:, FL:], (0, 2, 1)).reshape(B, N * TL)
    wrap = lists.reshape(B, 8, FL // 16, 16)
    idx16 = np.ascontiguousarray(
        np.transpose(wrap, (0, 1, 3, 2)).reshape(B, 128, FL // 16)
    ).astype(np.int16)

    w1f = np.transpose(np.asarray(W1, np.float32), (1, 2, 0)).reshape(CN, K)

    def eo(Wm):  # [K_out, K_in, N] -> even/odd lhsT [(n,kp), K_out] bf16
        Wm = np.asarray(Wm, np.float32)
        we = np.transpose(Wm[:, 0::2, :], (2, 1, 0)).reshape(N * KP, K)
        wo = np.transpose(Wm[:, 1::2, :], (2, 1, 0)).reshape(N * KP, K)
        return (np.ascontiguousarray(we).astype(bf16),
                np.ascontiguousarray(wo).astype(bf16))

    w2e, w2o = eo(W2)
    w3e, w3o = eo(W3)

    # Replication matrices over all 8 groups: repe[q, (g,kp)] = (q == 2*kp)
    q = np.arange(K)[:, None]
    p = np.arange(128)[None, :]
    repe = (q == 2 * (p % KP)).astype(bf16)
    repo = (q == 2 * (p % KP) + 1).astype(bf16)

    # fc1 weights: [H1, K*F] -> [K, FG, H1] zero-padded, per-core k-slice.
    fc1 = np.asarray(fc1_w, np.float32).reshape(H1, K, F)
    fc1t = np.zeros((K, FG, H1), dtype=bf16)
    fc1t[:, :F] = np.transpose(fc1, (1, 2, 0)).astype(bf16)

    fc2wt = np.ascontiguousarray(np.asarray(fc2_w, np.float32).T)  # [H1, H2]
    fcowt = np.ascontiguousarray(np.asarray(fco_w, np.float32).T)  # [H2, NCLS]

    def col(v, n):
        return np.asarray(v, np.float32).reshape(n, 1)

    shared = dict(
        w1=w1f.astype(bf16), w2e=w2e, w2o=w2o, w3e=w3e, w3o=w3o,
        repe=repe, repo=repo,
        fc1b=col(fc1_b, H1), bn1g=col(bn1_g, H1), bn1b=col(bn1_b, H1),
        fc2wt=fc2wt, fc2b=col(fc2_b, H2), bn2g=col(bn2_g, H2),
        bn2b=col(bn2_b, H2), fcowt=fcowt, fcob=col(fco_b, NCLS),
    )

    in_maps = []
    for c in range(cfg.ncores):
        bsl = slice(c * BL, (c + 1) * BL)
        fc1wt_c = np.ascontiguousarray(
            fc1t[c * KL:(c + 1) * KL].reshape(KL * FG, H1))
        m = dict(shared)
        m.update(
            xt=np.ascontiguousarray(xt[bsl]),
            idx16=np.ascontiguousarray(idx16[bsl]),
            fc1wt=fc1wt_c,
        )
        in_maps.append(m)
    return in_maps


def postprocess(out_dev: np.ndarray, cfg: Cfg = CFG) -> np.ndarray:
    """Device out columns are (sample-within-core, core) ordered; return
    [B, NCLS] in global sample order (core-major)."""
    o = np.asarray(out_dev, np.float32).reshape(cfg.NCLS, cfg.BL, cfg.ncores)
    return np.ascontiguousarray(o.transpose(2, 1, 0).reshape(cfg.B, cfg.NCLS))


# ---------------------------------------------------------------------------
# Device program
# ---------------------------------------------------------------------------

def build_program(cfg: Cfg):
    import concourse.bass as bass  # noqa: F401
    import concourse.bacc as bacc
    import concourse.mybir as mybir
    import concourse.tile as tile
    from concourse.masks import make_identity

    dt = mybir.dt.float32
    bf = mybir.dt.bfloat16
    u32 = mybir.dt.uint32
    i16 = mybir.dt.int16
    B, C, N, K, FG, FL = cfg.B, cfg.C, cfg.N, cfg.K, cfg.FG, cfg.FL
    BL, CN, KL, KP, CHT, TL = (cfg.BL, cfg.CN, cfg.KL, cfg.KP, cfg.CHT,
                               cfg.TL)
    H1, H2, NCLS = cfg.H1, cfg.H2, cfg.NCLS
    CHUNK, SEGS, WCOL = cfg.CHUNK, cfg.SEGS, cfg.WCOL
    SPLIT = cfg.SPLIT
    NCORES = cfg.ncores
    SEGMAX = max(w for _, w in SEGS)
    BLK = 1024
    rg = [list(range(NCORES))]

    nc = bacc.Bacc("TRN2", target_bir_lowering=False, debug=False,
                   num_devices=NCORES, num_swdge_queues=4)

    xt = nc.dram_tensor("xt", [BL, CN, FG], bf, kind="ExternalInput")
    idx16 = nc.dram_tensor("idx16", [BL, 128, WCOL], i16,
                           kind="ExternalInput")
    w1 = nc.dram_tensor("w1", [CN, K], bf, kind="ExternalInput")
    w2e = nc.dram_tensor("w2e", [CHT, K], bf, kind="ExternalInput")
    w2o = nc.dram_tensor("w2o", [CHT, K], bf, kind="ExternalInput")
    w3e = nc.dram_tensor("w3e", [CHT, K], bf, kind="ExternalInput")
    w3o = nc.dram_tensor("w3o", [CHT, K], bf, kind="ExternalInput")
    repe = nc.dram_tensor("repe", [K, 128], bf, kind="ExternalInput")
    repo = nc.dram_tensor("repo", [K, 128], bf, kind="ExternalInput")
    fc1wt = nc.dram_tensor("fc1wt", [KL * FG, H1], bf, kind="ExternalInput")
    fc1b = nc.dram_tensor("fc1b", [H1, 1], dt, kind="ExternalInput")
    bn1g = nc.dram_tensor("bn1g", [H1, 1], dt, kind="ExternalInput")
    bn1b = nc.dram_tensor("bn1b", [H1, 1], dt, kind="ExternalInput")
    fc2wt = nc.dram_tensor("fc2wt", [H1, H2], dt, kind="ExternalInput")
    fc2b = nc.dram_tensor("fc2b", [H2, 1], dt, kind="ExternalInput")
    bn2g = nc.dram_tensor("bn2g", [H2, 1], dt, kind="ExternalInput")
    bn2b = nc.dram_tensor("bn2b", [H2, 1], dt, kind="ExternalInput")
    fcowt = nc.dram_tensor("fcowt", [H2, NCLS], dt, kind="ExternalInput")
    fcob = nc.dram_tensor("fcob", [NCLS, 1], dt, kind="ExternalInput")
    out = nc.dram_tensor("out", [NCLS, B], dt, kind="ExternalOutput")

    def tail_pieces():
        """(n, seg_idx, seg_local_start, width, tail_local_start) pieces
        covering each neighbour's [n*TL, (n+1)*TL) slice of group-7's
        entry list, split at gather-segment boundaries."""
        out_runs = []
        for n in range(N):
            e0, e1 = n * TL, (n + 1) * TL
            for si, (s0, slen) in enumerate(SEGS):
                lo = max(e0, s0)
                hi = min(e1, s0 + slen)
                if lo < hi:
                    out_runs.append((n, si, lo - s0, hi - lo, lo - e0))
        return out_runs

    with tile.TileContext(nc) as tc:
        with (
            tc.tile_pool(name="consts", bufs=1) as consts,
            tc.tile_pool(name="xcp", bufs=2) as xcp,
            tc.tile_pool(name="idxp", bufs=3) as idxp,
            tc.tile_pool(name="tabs", bufs=3) as tabsp,
            tc.tile_pool(name="gop", bufs=4) as gop,
            tc.tile_pool(name="tbp", bufs=1) as tbp,
            tc.tile_pool(name="hp", bufs=1) as hp,
            tc.tile_pool(name="hst", bufs=2) as hstp,
            tc.tile_pool(name="work", bufs=2) as work,
            tc.tile_pool(name="xbp", bufs=2) as xbp,
            tc.tile_pool(name="dram", bufs=1, space="DRAM") as dram,
        ):
            # ---- constants needed by conv1(0) go first; the rest are
            # emitted after the first gathers so they don't delay the head.
            w1_t = consts.tile([CN, K], bf)
            nc.sync.dma_start(w1_t[:], w1[:])
            repe_t = consts.tile([K, 128], bf)
            nc.sync.dma_start(repe_t[:], repe[:])
            repo_t = consts.tile([K, 128], bf)
            nc.sync.dma_start(repo_t[:], repo[:])

            bounce = dram.tile([BL, NCORES, KL, FG], bf)
            recv = dram.tile([BL, NCORES, KL, FG], bf)
            # sample-7 AllToAll face-range splits (seg0 / seg1 / seg2+tail)
            S7R = [(0, SEGS[1][0]), (SEGS[1][0], SEGS[2][0]),
                   (SEGS[2][0], FG)]
            bounce7 = [dram.tile([NCORES, KL, hi - lo], bf,
                                 name=f"bounce7_{i}")
                       for i, (lo, hi) in enumerate(S7R)]
            recv7 = [dram.tile([NCORES, KL, hi - lo], bf,
                               name=f"recv7_{i}")
                     for i, (lo, hi) in enumerate(S7R)]
            y1snd1 = dram.tile([H1, 32], dt)
            y1rcv1 = dram.tile([H1, 32], dt)
            y1snd2 = dram.tile([H1, 32], dt)
            y1rcv2 = dram.tile([H1, 32], dt)

            with (
                tc.tile_pool(name="cpsum", bufs=2, space="PSUM") as cpsum,
                tc.tile_pool(name="rpsum", bufs=2, space="PSUM") as rpsum,
                tc.tile_pool(name="fpsum", bufs=1, space="PSUM") as fpsum,
            ):
                def build_table(tab, hs, f0, w):
                    """REP-matmul an h chunk [K, w] into the packed table."""
                    tb = tab[:].bitcast(bf).rearrange(
                        "p (f two) -> p f two", two=2)
                    pse = rpsum.tile([128, CHUNK], dt, tag="rp")
                    nc.tensor.matmul(out=pse[:, :w], lhsT=repe_t[:],
                                     rhs=hs[:, :w], start=True, stop=True)
                    nc.vector.tensor_copy(tb[:, f0:f0 + w, 0], pse[:, :w])
                    pso = rpsum.tile([128, CHUNK], dt, tag="rp")
                    nc.tensor.matmul(out=pso[:, :w], lhsT=repo_t[:],
                                     rhs=hs[:, :w], start=True, stop=True)
                    nc.scalar.copy(tb[:, f0:f0 + w, 1], pso[:, :w])

                st = {}  # per-sample tile state

                def load_idx(s):
                    idx_t = idxp.tile([128, WCOL], i16, tag="it")
                    nc.sync.dma_start(idx_t[:], idx16[s])
                    st[s] = dict(idx=idx_t)

                XBLK = 2252  # x staged in 4 big DMAs instead of 18 small

                def conv1_full(s, tab1):
                    st[s]["tab1"] = tab1
                    for xb0, xbw in _chunks(0, FG, XBLK):
                        xc = xbp.tile([CN, XBLK], bf, tag="xb")
                        nc.sync.dma_start(xc[:, :xbw],
                                          xt[s, :, xb0:xb0 + xbw])
                        for f0, w in _chunks(xb0, xbw, CHUNK):
                            lo = f0 - xb0
                            ps = cpsum.tile([K, CHUNK], dt, tag="cp")
                            nc.tensor.matmul(out=ps[:, :w], lhsT=w1_t[:],
                                             rhs=xc[:, lo:lo + w],
                                             start=True, stop=True)
                            hs = hstp.tile([K, CHUNK], bf, tag="hst")
                            nc.vector.tensor_copy(hs[:, :w], ps[:, :w])
                            build_table(tab1, hs, f0, w)

                def gather_seg(s, tab_key, seg):
                    s0, slen = SEGS[seg]
                    go = gop.tile([128, SEGMAX], u32, tag="go")
                    nc.gpsimd.ap_gather(
                        out_ap=go[:, :slen], in_ap=st[s][tab_key][:],
                        idxs_ap=st[s]["idx"][:, s0 // 16:(s0 + slen) // 16],
                        channels=128, num_elems=FG, d=1, num_idxs=slen)
                    return go

                def stage_tail(gos):
                    """SBUF->SBUF DMA group-7's gathered entries into the
                    main (n, kp) partition layout: tailbuf[16n+kp, u] =
                    go[112+kp, n*TL+u]. Returns the [CHT, TL] u32 tile."""
                    tb = tbp.tile([CHT, TL], u32, tag="tb")
                    for n, si, lo, rw, u0 in tail_pieces():
                        nc.sync.dma_start(
                            tb[16 * n:16 * n + 16, u0:u0 + rw],
                            gos[si][112:128, lo:lo + rw])
                    return tb

                def conv_segs(gos, we_t, wo_t, sink, seg_ids):
                    """Conv faces covered by the given gather segments."""
                    for si in seg_ids:
                        s0, slen = SEGS[si]
                        gb = gos[si][:, :slen].bitcast(bf).rearrange(
                            "p (f two) -> p f two", two=2)
                        for f0, w in _chunks(s0, slen, CHUNK):
                            lo = f0 - s0
                            ps = cpsum.tile([K, CHUNK], dt, tag="cp")
                            nc.tensor.matmul(out=ps[:, :w],
                                             lhsT=we_t[:],
                                             rhs=gb[0:CHT, lo:lo + w, 0],
                                             start=True, stop=False)
                            nc.tensor.matmul(out=ps[:, :w],
                                             lhsT=wo_t[:],
                                             rhs=gb[0:CHT, lo:lo + w, 1],
                                             start=False, stop=True)
                            sink(f0, w, ps)

                def conv_tail(gos, we_t, wo_t, sink):
                    """Conv the tail faces [FL, FG) via the restaged
                    group-7 entries."""
                    if FL >= FG:
                        return
                    tb = stage_tail(gos)
                    tbb = tb[:].bitcast(bf).rearrange(
                        "p (f two) -> p f two", two=2)
                    for f0, w in _chunks(FL, FG - FL, CHUNK):
                        lo = f0 - FL
                        ps = cpsum.tile([K, CHUNK], dt, tag="cp")
                        nc.tensor.matmul(out=ps[:, :w], lhsT=we_t[:],
                                         rhs=tbb[:, lo:lo + w, 0],
                                         start=True, stop=False)
                        nc.tensor.matmul(out=ps[:, :w], lhsT=wo_t[:],
                                         rhs=tbb[:, lo:lo + w, 1],
                                         start=False, stop=True)
                        sink(f0, w, ps)

                def conv2_full(s, gos, tab2):
                    st[s]["tab2"] = tab2

                    def sink(f0, w, ps):
                        hs = hstp.tile([K, CHUNK], bf, tag="hst")
                        nc.vector.tensor_copy(hs[:, :w], ps[:, :w])
                        build_table(tab2, hs, f0, w)
                    conv_segs(gos, w2e_t, w2o_t, sink, [0, 1, 2])
                    conv_tail(gos, w2e_t, w2o_t, sink)

                def conv3_sink(s):
                    hs3 = hp.tile([K, FG], bf, tag="h3", name=f"h3_{s}")

                    def sink(f0, w, ps):
                        nc.vector.tensor_copy(hs3[:, f0:f0 + w], ps[:, :w])
                    return hs3, sink

                def conv3_full(s, gos):
                    hs3, sink = conv3_sink(s)
                    conv_segs(gos, w3e_t, w3o_t, sink, [0, 1, 2])
                    conv_tail(gos, w3e_t, w3o_t, sink)
                    # single bounce write per sample: the AllToAll's input
                    # must have one writer (chunked writers race the
                    # collective on HW).
                    nc.sync.dma_start(bounce[s], hs3[:])

                def cc_sample(s):
                    nc.gpsimd.collective_compute(
                        "AllToAll", mybir.AluOpType.bypass,
                        replica_groups=rg,
                        ins=[bounce[s].opt()], outs=[recv[s].opt()])

                # ---- fc1: y1ps[:, cols] += fc1wt.T @ transposed recv rows,
                # accumulated per column group over KL x 71 PSUM chunks.
                y1ps = fpsum.tile([H1, B], dt, tag="y1")
                fc1_state = {}

                def fc1_part(grp, c0, ncols, bblocks, load_rows, total_nst):
                    stt = fc1_state.setdefault(grp, dict(stp=0))
                    for kl in range(KL):
                        for b0, bw in bblocks:
                            lt_in = work.tile([ncols, BLK], bf,
                                              tag=f"ltin{ncols}")
                            load_rows(lt_in, kl, b0, bw)
                            r0 = kl * FG + b0
                            nfull = bw // 128
                            wt = work.tile([128, (BLK // 128) * H1], bf,
                                           tag="fw")
                            if nfull:
                                nc.sync.dma_start(
                                    wt[:, :nfull * H1].rearrange(
                                        "p (c h) -> p c h", h=H1),
                                    fc1wt[r0:r0 + nfull * 128, :].rearrange(
                                        "(c p) h -> p c h", p=128))
                            for ci, (s0c, wc) in enumerate(
                                    _chunks(0, bw, 128)):
                                pst = rpsum.tile([128, 16], bf, tag="tT")
                                nc.tensor.transpose(
                                    pst[:wc, :ncols],
                                    lt_in[:, s0c:s0c + wc],
                                    identB[:ncols, :ncols])
                                ltt = work.tile([128, ncols], bf,
                                                tag=f"ltt{ncols}")
                                nc.vector.tensor_copy(ltt[:wc, :],
                                                      pst[:wc, :ncols])
                                if wc == 128:
                                    lhsT = wt[:, ci * H1:(ci + 1) * H1]
                                else:
                                    wtp = work.tile([128, H1], bf, tag="fwp")
                                    nc.sync.dma_start(
                                        wtp[:wc, :],
                                        fc1wt[r0 + s0c:r0 + s0c + wc, :])
                                    lhsT = wtp[:wc, :]
                                nc.tensor.matmul(
                                    out=y1ps[:, c0:c0 + ncols],
                                    lhsT=lhsT, rhs=ltt[:wc, :],
                                    start=(stt["stp"] == 0),
                                    stop=(stt["stp"] == total_nst - 1))
                                stt["stp"] += 1

                FULL_BLOCKS = _chunks(0, FG, BLK)
                NST_FULL = KL * sum(len(_chunks(0, bw, 128))
                                    for _, bw in FULL_BLOCKS)

                def fc1_half(h):
                    def load_rows(t, kl, b0, bw):
                        nc.sync.dma_start(
                            t[:, :bw], recv[2 * h:2 * h + 2, :, kl,
                                            b0:b0 + bw])
                    fc1_part(f"h{h}", 16 * h, 16, FULL_BLOCKS, load_rows,
                             NST_FULL)

                BLOCKS_7 = [_chunks(lo, hi - lo, BLK) for lo, hi in S7R]
                NST_P7 = KL * sum(len(_chunks(0, bw, 128))
                                  for blocks in BLOCKS_7
                                  for _, bw in blocks)

                def fc1_half3(part):
                    """cols 48:64 = samples 6,7; face-range `part` so each
                    part unblocks as soon as its sample-7 AllToAll lands."""
                    src, off = recv7[part], S7R[part][0]

                    def load_rows(t, kl, b0, bw):
                        nc.sync.dma_start(
                            t[0:8, :bw], recv[6:7, :, kl, b0:b0 + bw])
                        nc.sync.dma_start(
                            t[8:16, :bw],
                            src[:, kl, b0 - off:b0 - off + bw])
                    fc1_part("h3", 48, 16, BLOCKS_7[part], load_rows, NST_P7)

                def new_tab(kind, s):
                    return tabsp.tile([128, FG], u32, tag="tab",
                                      name=f"tab{kind}_{s}")

                def a2a(src, dst):
                    nc.gpsimd.collective_compute(
                        "AllToAll", mybir.AluOpType.bypass,
                        replica_groups=rg,
                        ins=[src[:].opt()], outs=[dst[:].opt()])

                def allreduce(src, dst):
                    nc.gpsimd.collective_compute(
                        "AllReduce", mybir.AluOpType.add, replica_groups=rg,
                        ins=[src[:].opt()], outs=[dst[:].opt()])

                # ---- prologue: sample-0 table + first gathers before all
                # other constants, so the head is just idx0+conv1(0).
                load_idx(0)
                # keep the (otherwise unused) xcp pool's footprint so the
                # SBUF layout of the pools behind it doesn't shift — the
                # ap_gather ucode rate is sensitive to table placement.
                xpad0 = xcp.tile([CN, CHUNK], bf, tag="xc")
                nc.vector.memset(xpad0[:, :8], 0.0)
                xpad1 = xcp.tile([CN, CHUNK], bf, tag="xc")
                nc.vector.memset(xpad1[:, :8], 0.0)
                conv1_full(0, new_tab(1, 0))
                gos0 = [gather_seg(0, "tab1", i) for i in range(3)]
                load_idx(1)
                conv1_full(1, new_tab(1, 1))

                w2e_t = consts.tile([CHT, K], bf)
                nc.sync.dma_start(w2e_t[:], w2e[:])
                w2o_t = consts.tile([CHT, K], bf)
                nc.sync.dma_start(w2o_t[:], w2o[:])
                w3e_t = consts.tile([CHT, K], bf)
                nc.sync.dma_start(w3e_t[:], w3e[:])
                w3o_t = consts.tile([CHT, K], bf)
                nc.sync.dma_start(w3o_t[:], w3o[:])
                identB = consts.tile([B, B], bf)
                make_identity(nc, identB)
                zcol = consts.tile([128, 1], dt)
                nc.vector.memset(zcol[:], 0.0)

                conv2_full(0, gos0, new_tab(2, 0))

                # ---- software-pipelined sample loop ----
                for s in range(BL):
                    nxt = s + 1 < BL
                    # table slot rotation in conv1-then-conv2 order (the
                    # 3-slot cycle then always lands writers on slots whose
                    # readers finished an iteration ago)
                    t1n = new_tab(1, s + 2) if s + 2 < BL else None
                    t2n = new_tab(2, s + 1) if nxt else None
                    if nxt:
                        gos2 = [gather_seg(s + 1, "tab1", i)
                                for i in range(3)]
                    if s >= 1:
                        cc_sample(s - 1)
                    if s < BL - 1:
                        gos3 = [gather_seg(s, "tab2", i) for i in range(3)]
                        if s + 2 < BL:
                            load_idx(s + 2)
                            conv1_full(s + 2, t1n)
                        if nxt:
                            conv2_full(s + 1, gos2, t2n)
                        conv3_full(s, gos3)
                        if s == 3:
                            fc1_half(0)
                        elif s == 5:
                            fc1_half(1)
                        elif s == 6:
                            # AllReduce of halves 0,1 (cols 0:32): fired
                            # inside the loop, off the tail critical path
                            y1l1 = work.tile([H1, 32], dt, tag="y1l1")
                            nc.vector.tensor_copy(y1l1[:], y1ps[:, 0:32])
                            nc.sync.dma_start(y1snd1[:], y1l1[:])
                            allreduce(y1snd1, y1rcv1)
                    else:
                        # ---- last sample: half2 fills iter-7's PE slack;
                        # the AllToAll is split in 3 face ranges so the
                        # exchange and fc1 half3 overlap the final gathers.
                        g3a = gather_seg(s, "tab2", 0)
                        g3b = gather_seg(s, "tab2", 1)
                        hs3, sink = conv3_sink(s)
                        conv_segs([g3a], w3e_t, w3o_t, sink, [0])
                        nc.sync.dma_start(bounce7[0][:],
                                          hs3[:, S7R[0][0]:S7R[0][1]])
                        fc1_half(2)
                        g3c = gather_seg(s, "tab2", 2)
                        gos3 = [g3a, g3b, g3c]
                        # triggers sit after all gathers in the Pool queue:
                        # the sequencer runs ahead of the gather engine, so
                        # each fires as soon as its bounce DMA lands.
                        a2a(bounce7[0], recv7[0])
                        fc1_half3(0)
                        conv_segs(gos3, w3e_t, w3o_t, sink, [1])
                        nc.sync.dma_start(bounce7[1][:],
                                          hs3[:, S7R[1][0]:S7R[1][1]])
                        a2a(bounce7[1], recv7[1])
                        fc1_half3(1)
                        conv_segs(gos3, w3e_t, w3o_t, sink, [2])
                        conv_tail(gos3, w3e_t, w3o_t, sink)
                        nc.sync.dma_start(bounce7[2][:],
                                          hs3[:, S7R[2][0]:S7R[2][1]])
                        a2a(bounce7[2], recv7[2])
                        fc1_half3(2)
                        y1l2 = work.tile([H1, 32], dt, tag="y1l2")
                        nc.vector.tensor_copy(y1l2[:], y1ps[:, 32:64])
                        nc.sync.dma_start(y1snd2[:], y1l2[:])
                        allreduce(y1snd2, y1rcv2)
                    st.pop(s)

                # ---- head (replicated) ----
                def bn_relu(y, h, g_ap, b_ap, relu=True):
                    """In-place batchnorm(+relu) on SBUF tile y [h, B]."""
                    mean = work.tile([h, 1], dt, tag=f"bn_m{h}")
                    nc.vector.reduce_sum(mean[:], y[:],
                                         axis=mybir.AxisListType.X)
                    nc.vector.tensor_scalar_mul(mean[:], mean[:], 1.0 / B)
                    sq = work.tile([h, B], dt, tag=f"bn_sq{h}")
                    nc.vector.tensor_tensor(out=sq[:], in0=y[:], in1=y[:],
                                            op=mybir.AluOpType.mult)
                    var = work.tile([h, 1], dt, tag=f"bn_v{h}")
                    nc.vector.reduce_sum(var[:], sq[:],
                                         axis=mybir.AxisListType.X)
                    nc.vector.tensor_scalar_mul(var[:], var[:], 1.0 / B)
                    m2 = work.tile([h, 1], dt, tag=f"bn_m2{h}")
                    nc.vector.tensor_tensor(out=m2[:], in0=mean[:],
                                            in1=mean[:],
                                            op=mybir.AluOpType.mult)
                    nc.vector.tensor_tensor(out=var[:], in0=var[:], in1=m2[:],
                                            op=mybir.AluOpType.subtract)
                    nc.vector.tensor_scalar_add(var[:], var[:], cfg.EPS)
                    std = work.tile([h, 1], dt, tag=f"bn_s{h}")
                    nc.scalar.activation(std[:], var[:],
                                         mybir.ActivationFunctionType.Sqrt,
                                         bias=zcol[:h, :1])
                    rstd = work.tile([h, 1], dt, tag=f"bn_r{h}")
                    nc.vector.reciprocal(rstd[:], std[:])
                    gl = work.tile([h, 1], dt, tag=f"bn_g{h}")
                    nc.sync.dma_start(gl[:], g_ap[:])
                    bl = work.tile([h, 1], dt, tag=f"bn_b{h}")
                    nc.sync.dma_start(bl[:], b_ap[:])
                    scale = work.tile([h, 1], dt, tag=f"bn_sc{h}")
                    nc.vector.tensor_tensor(out=scale[:], in0=rstd[:],
                                            in1=gl[:],
                                            op=mybir.AluOpType.mult)
                    shift = work.tile([h, 1], dt, tag=f"bn_sh{h}")
                    nc.vector.tensor_tensor(out=shift[:], in0=mean[:],
                                            in1=scale[:],
                                            op=mybir.AluOpType.mult)
                    nc.vector.tensor_tensor(out=shift[:], in0=bl[:],
                                            in1=shift[:],
                                            op=mybir.AluOpType.subtract)
                    nc.vector.tensor_scalar(
                        out=y[:], in0=y[:], scalar1=scale[:], scalar2=shift[:],
                        op0=mybir.AluOpType.mult, op1=mybir.AluOpType.add)
                    if relu:
                        nc.scalar.activation(y[:], y[:],
                                             mybir.ActivationFunctionType.Relu,
                                             bias=zcol[:h, :1])

                y1 = work.tile([H1, B], dt, tag="y1h")
                nc.sync.dma_start(y1[:, 0:32], y1rcv1[:])
                nc.sync.dma_start(y1[:, 32:64], y1rcv2[:])
                f1b = work.tile([H1, 1], dt, tag="f1b")
                nc.sync.dma_start(f1b[:], fc1b[:])
                nc.vector.tensor_scalar_add(y1[:], y1[:], f1b[:])
                bn_relu(y1, H1, bn1g, bn1b)

                w2f = work.tile([H1, H2], dt, tag="w2f")
                nc.sync.dma_start(w2f[:], fc2wt[:])
                ps2 = cpsum.tile([K, CHUNK], dt, tag="cp")
                nc.tensor.matmul(out=ps2[0:H2, 0:B], lhsT=w2f[:], rhs=y1[:],
                                 start=True, stop=True)
                y2 = work.tile([H2, B], dt, tag="y2h")
                nc.vector.tensor_copy(y2[:], ps2[0:H2, 0:B])
                f2b = work.tile([H2, 1], dt, tag="f2b")
                nc.sync.dma_start(f2b[:], fc2b[:])
                nc.vector.tensor_scalar_add(y2[:], y2[:], f2b[:])
                bn_relu(y2, H2, bn2g, bn2b)

                wof = work.tile([H2, NCLS], dt, tag="wof")
                nc.sync.dma_start(wof[:], fcowt[:])
                pso = cpsum.tile([K, CHUNK], dt, tag="cp")
                nc.tensor.matmul(out=pso[0:NCLS, 0:B], lhsT=wof[:], rhs=y2[:],
                                 start=True, stop=True)
                yo = work.tile([NCLS, B], dt, tag="yo")
                nc.vector.tensor_copy(yo[:], pso[0:NCLS, 0:B])
                fob = work.tile([NCLS, 1], dt, tag="fob")
                nc.sync.dma_start(fob[:], fcob[:])
                nc.vector.tensor_scalar_add(yo[:], yo[:], fob[:])
                nc.sync.dma_start(out[:], yo[:])

    nc.compile()
    return nc


_CACHE: dict = {}


def _get_program(cfg: Cfg):
    key = cfg
    if key not in _CACHE:
        _CACHE[key] = build_program(cfg)
    return _CACHE[key]


def kernel(**inputs) -> np.ndarray:
    from concourse import bass_utils

    cfg = CFG
    nc = _get_program(cfg)
    in_maps = prep_core_inputs(cfg, **inputs)
    res = bass_utils.run_bass_kernel_spmd(
        nc, in_maps, core_ids=list(range(cfg.ncores)))
    return postprocess(res.results[0]["out"], cfg)


# revision 21
# speedup vs baseline: 1.1961x; 1.0007x over previous
"""Trainium2 Bass kernel for nn_CNN_9818295238933 (gnn_message_passing).

Data-parallel over batch across 8 cores (8 samples each). Per sample:
  conv1 (PE, bf16) -> h1 [32, F] -> REP matmul replicates h1 across 8
  partition groups as a bf16-pair-packed SBUF table [128, F] (partition
  (g, kp) holds the bf16 pair (h[2kp], h[2kp+1]) at face f).
  ap_gather (GPSIMD, SBUF-local) gathers the table with that sample's
  adjacency: groups 0-6 carry neighbour slot n for faces [0, FL); group 7
  carries the tail faces [FL, FG) of all 7 slots concatenated, so all 8
  Q7 cores work and each instruction processes FL/seg indices instead of
  FG. Gathered tiles feed the next conv directly as strided bf16 matmul
  rhs (contraction over (n, kp) partitions, even/odd k accumulated in
  PSUM); tail faces get per-n 16-partition matmuls from group 7's slice.
  Repeat for conv2 -> table2 -> gather -> conv3.

The ap_gather ucode is the hard bottleneck (~27.3 ns/index per Q7 core,
measured on idle HW; 48 segment-gathers x ~71.3 us = 3.42 ms). The
sample loop is software-pipelined so the Pool engine never waits:
gathers are emitted as [g2(s+1) segs][cc(s-1)][g3(s) segs]; convs are
emitted consumer-first (conv2(s+1), conv3(s) BEFORE conv1(s+2)) so PE
frees gather buffers promptly. All three tables (tab1 x2 live +
tab2 x1) share one tag-rotated 3-slot pool, which double-buffers tab1
at startup (kills the prologue stall) while keeping baseline SBUF use.

Head: idx(0)+conv1(0) emitted before all other constants. Tail: h3
bounces to DRAM per sample and a per-sample AllToAll redistributes
k-slices; fc1 runs as 3 16-col halves (iters 3/5/6) + sample-6 and
sample-7 8-col pieces; sample 7's AllToAll is split at face 2624 so
its first third (and fc1 piece7A) overlaps the final gather segment;
AllReduce is split in 3 column groups (2 fire during the loop).
BN+ReLU+fc2+BN+ReLU+fco replicated.

Self-contained: hardcodes all shapes; only imports the Trainium toolchain.
"""

import sys
from dataclasses import dataclass

if "/opt/trn_rl_repo" not in sys.path:
    sys.path.insert(0, "/opt/trn_rl_repo")

import numpy as np


@dataclass(frozen=True)
class Cfg:
    ncores: int = 8
    B: int = 64
    C: int = 12
    N: int = 7
    K: int = 32
    F: int = 9000
    FG: int = 9008          # compute/table extent (F padded to mult of 16)
    FL: int = 7888          # main faces per neighbour group (= FG * 7/8 pad16)
    H1: int = 100
    H2: int = 30
    NCLS: int = 2
    EPS: float = 1e-5
    CHUNK: int = 512        # PSUM f-chunk

    @property
    def BL(self):
        return self.B // self.ncores

    @property
    def CN(self):
        return self.C * self.N

    @property
    def KL(self):
        return self.K // self.ncores

    @property
    def KP(self):
        return self.K // 2

    @property
    def CHT(self):
        return self.N * self.KP  # 112 main channels

    @property
    def TL(self):
        return self.FG - self.FL  # 1120 tail faces

    @property
    def SEGS(self):
        # Segment starts must be multiples of 32 entries: the gather
        # ucode reads the wrapped idx list as u32 words, and a 2-byte
        # misaligned base corrupts words crossing 16-byte boundaries.
        if self.FL == 9008:  # tail disabled
            return [(0, 3008), (3008, 3008), (6016, 2992)]
        return [(0, 2624), (2624, 2624), (5248, 2640)]

    @property
    def WCOL(self):
        return self.FL // 16  # wrapped idx columns (493)

    @property
    def SPLIT(self):
        return self.SEGS[1][0]  # face split for sample-7's early AllToAll


CFG = Cfg()


def _chunks(f0, flen, step):
    out = []
    f = f0
    while f < f0 + flen:
        out.append((f, min(step, f0 + flen - f)))
        f += step
    return out


# ---------------------------------------------------------------------------
# Host-side input preparation
# ---------------------------------------------------------------------------

def prep_core_inputs(cfg: Cfg, x, adjacencies, W1, W2, W3, fc1_w, fc1_b, bn1_g,
                     bn1_b, fc2_w, fc2_b, bn2_g, bn2_b, fco_w, fco_b):
    import ml_dtypes
    bf16 = ml_dtypes.bfloat16

    B, C, N, K, F, FG, FL = (cfg.B, cfg.C, cfg.N, cfg.K, cfg.F, cfg.FG,
                             cfg.FL)
    BL, CN, KL, KP, TL = cfg.BL, cfg.CN, cfg.KL, cfg.KP, cfg.TL
    H1, H2, NCLS = cfg.H1, cfg.H2, cfg.NCLS

    x = np.asarray(x, dtype=np.float32)
    adj = np.asarray(adjacencies).astype(np.int64)[:, 0]  # [B, F, N]

    # x [B, C, F, N] -> xt [B, (c,n), FG] bf16, zero-padded along f.
    xt = np.zeros((B, CN, FG), dtype=bf16)
    xt[:, :, :F] = np.transpose(x, (0, 1, 3, 2)).reshape(B, CN, F).astype(bf16)

    # Gather index lists, one per 16-partition group:
    #   group n < 7: adj[b, f, n] for f in [0, FL)
    #   group 7:     adj[b, FL+u, n] at position n*TL+u (pad to FL with 0)
    # wrapped so entry i sits at [16g + i%16, i//16]. Segment boundaries
    # are multiples of 16 so column-slicing yields each segment's list.
    idx_pad = np.zeros((B, FG, N), dtype=np.int64)
    idx_pad[:, :F] = adj
    lists = np.zeros((B, 8, FL), dtype=np.int64)
    lists[:, :7, :] = np.transpose(idx_pad[:, :FL], (0, 2, 1))
    lists[:, 7, :N * TL] = np.transpose(
        idx_pad[:, FL:], (0, 2, 1)).reshape(B, N * TL)
    wrap = lists.reshape(B, 8, FL // 16, 16)
    idx16 = np.ascontiguousarray(
        np.transpose(wrap, (0, 1, 3, 2)).reshape(B, 128, FL // 16)
    ).astype(np.int16)

    w1f = np.transpose(np.asarray(W1, np.float32), (1, 2, 0)).reshape(CN, K)

    def eo(Wm):  # [K_out, K_in, N] -> even/odd lhsT [(n,kp), K_out] bf16
        Wm = np.asarray(Wm, np.float32)
        we = np.transpose(Wm[:, 0::2, :], (2, 1, 0)).reshape(N * KP, K)
        wo = np.transpose(Wm[:, 1::2, :], (2, 1, 0)).reshape(N * KP, K)
        return (np.ascontiguousarray(we).astype(bf16),
                np.ascontiguousarray(wo).astype(bf16))

    w2e, w2o = eo(W2)
    w3e, w3o = eo(W3)

    # Replication matrices over all 8 groups: repe[q, (g,kp)] = (q == 2*kp)
    q = np.arange(K)[:, None]
    p = np.arange(128)[None, :]
    repe = (q == 2 * (p % KP)).astype(bf16)
    repo = (q == 2 * (p % KP) + 1).astype(bf16)

    # fc1 weights: [H1, K*F] -> [K, FG, H1] zero-padded, per-core k-slice.
    fc1 = np.asarray(fc1_w, np.float32).reshape(H1, K, F)
    fc1t = np.zeros((K, FG, H1), dtype=bf16)
    fc1t[:, :F] = np.transpose(fc1, (1, 2, 0)).astype(bf16)

    fc2wt = np.ascontiguousarray(np.asarray(fc2_w, np.float32).T)  # [H1, H2]
    fcowt = np.ascontiguousarray(np.asarray(fco_w, np.float32).T)  # [H2, NCLS]

    def col(v, n):
        return np.asarray(v, np.float32).reshape(n, 1)

    shared = dict(
        w1=w1f.astype(bf16), w2e=w2e, w2o=w2o, w3e=w3e, w3o=w3o,
        repe=repe, repo=repo,
        fc1b=col(fc1_b, H1), bn1g=col(bn1_g, H1), bn1b=col(bn1_b, H1),
        fc2wt=fc2wt, fc2b=col(fc2_b, H2), bn2g=col(bn2_g, H2),
        bn2b=col(bn2_b, H2), fcowt=fcowt, fcob=col(fco_b, NCLS),
    )

    in_maps = []
    for c in range(cfg.ncores):
        bsl = slice(c * BL, (c + 1) * BL)
        fc1wt_c = np.ascontiguousarray(
            fc1t[c * KL:(c + 1) * KL].reshape(KL * FG, H1))
        m = dict(shared)
        m.update(
            xt=np.ascontiguousarray(xt[bsl]),
            idx16=np.ascontiguousarray(idx16[bsl]),
            fc1wt=fc1wt_c,
        )
        in_maps.append(m)
    return in_maps


def postprocess(out_dev: np.ndarray, cfg: Cfg = CFG) -> np.ndarray:
    """Device out columns are (sample-within-core, core) ordered; return
    [B, NCLS] in global sample order (core-major)."""
    o = np.asarray(out_dev, np.float32).reshape(cfg.NCLS, cfg.BL, cfg.ncores)
    return np.ascontiguousarray(o.transpose(2, 1, 0).reshape(cfg.B, cfg.NCLS))


# ---------------------------------------------------------------------------
# Device program
# ---------------------------------------------------------------------------

def build_program(cfg: Cfg):
    import concourse.bass as bass  # noqa: F401
    import concourse.bacc as bacc
    import concourse.mybir as mybir
    import concourse.tile as tile
    from concourse.masks import make_identity

    dt = mybir.dt.float32
    bf = mybir.dt.bfloat16
    u32 = mybir.dt.uint32
    i16 = mybir.dt.int16
    B, C, N, K, FG, FL = cfg.B, cfg.C, cfg.N, cfg.K, cfg.FG, cfg.FL
    BL, CN, KL, KP, CHT, TL = (cfg.BL, cfg.CN, cfg.KL, cfg.KP, cfg.CHT,
                               cfg.TL)
    H1, H2, NCLS = cfg.H1, cfg.H2, cfg.NCLS
    CHUNK, SEGS, WCOL = cfg.CHUNK, cfg.SEGS, cfg.WCOL
    SPLIT = cfg.SPLIT
    NCORES = cfg.ncores
    SEGMAX = max(w for _, w in SEGS)
    BLK = 1024
    rg = [list(range(NCORES))]

    nc = bacc.Bacc("TRN2", target_bir_lowering=False, debug=False,
                   num_devices=NCORES, num_swdge_queues=4)

    xt = nc.dram_tensor("xt", [BL, CN, FG], bf, kind="ExternalInput")
    idx16 = nc.dram_tensor("idx16", [BL, 128, WCOL], i16,
                           kind="ExternalInput")
    w1 = nc.dram_tensor("w1", [CN, K], bf, kind="ExternalInput")
    w2e = nc.dram_tensor("w2e", [CHT, K], bf, kind="ExternalInput")
    w2o = nc.dram_tensor("w2o", [CHT, K], bf, kind="ExternalInput")
    w3e = nc.dram_tensor("w3e", [CHT, K], bf, kind="ExternalInput")
    w3o = nc.dram_tensor("w3o", [CHT, K], bf, kind="ExternalInput")
    repe = nc.dram_tensor("repe", [K, 128], bf, kind="ExternalInput")
    repo = nc.dram_tensor("repo", [K, 128], bf, kind="ExternalInput")
    fc1wt = nc.dram_tensor("fc1wt", [KL * FG, H1], bf, kind="ExternalInput")
    fc1b = nc.dram_tensor("fc1b", [H1, 1], dt, kind="ExternalInput")
    bn1g = nc.dram_tensor("bn1g", [H1, 1], dt, kind="ExternalInput")
    bn1b = nc.dram_tensor("bn1b", [H1, 1], dt, kind="ExternalInput")
    fc2wt = nc.dram_tensor("fc2wt", [H1, H2], dt, kind="ExternalInput")
    fc2b = nc.dram_tensor("fc2b", [H2, 1], dt, kind="ExternalInput")
    bn2g = nc.dram_tensor("bn2g", [H2, 1], dt, kind="ExternalInput")
    bn2b = nc.dram_tensor("bn2b", [H2, 1], dt, kind="ExternalInput")
    fcowt = nc.dram_tensor("fcowt", [H2, NCLS], dt, kind="ExternalInput")
    fcob = nc.dram_tensor("fcob", [NCLS, 1], dt, kind="ExternalInput")
    out = nc.dram_tensor("out", [NCLS, B], dt, kind="ExternalOutput")

    def tail_pieces():
        """(n, seg_idx, seg_local_start, width, tail_local_start) pieces
        covering each neighbour's [n*TL, (n+1)*TL) slice of group-7's
        entry list, split at gather-segment boundaries."""
        out_runs = []
        for n in range(N):
            e0, e1 = n * TL, (n + 1) * TL
            for si, (s0, slen) in enumerate(SEGS):
                lo = max(e0, s0)
                hi = min(e1, s0 + slen)
                if lo < hi:
                    out_runs.append((n, si, lo - s0, hi - lo, lo - e0))
        return out_runs

    with tile.TileContext(nc) as tc:
        with (
            tc.tile_pool(name="consts", bufs=1) as consts,
            tc.tile_pool(name="xcp", bufs=2) as xcp,
            tc.tile_pool(name="idxp", bufs=3) as idxp,
            tc.tile_pool(name="tabs", bufs=3) as tabsp,
            tc.tile_pool(name="gop", bufs=4) as gop,
            tc.tile_pool(name="tbp", bufs=1) as tbp,
            tc.tile_pool(name="hp", bufs=1) as hp,
            tc.tile_pool(name="hst", bufs=2) as hstp,
            tc.tile_pool(name="work", bufs=2) as work,
            tc.tile_pool(name="xbp", bufs=2) as xbp,
            tc.tile_pool(name="dram", bufs=1, space="DRAM") as dram,
        ):
            # ---- constants needed by conv1(0) go first; the rest are
            # emitted after the first gathers so they don't delay the head.
            w1_t = consts.tile([CN, K], bf)
            nc.sync.dma_start(w1_t[:], w1[:])
            repe_t = consts.tile([K, 128], bf)
            nc.sync.dma_start(repe_t[:], repe[:])
            repo_t = consts.tile([K, 128], bf)
            nc.sync.dma_start(repo_t[:], repo[:])

            bounce = dram.tile([BL, NCORES, KL, FG], bf)
            recv = dram.tile([BL, NCORES, KL, FG], bf)
            # sample-7 AllToAll face-range splits (seg0 / seg1 / seg2+tail)
            S7R = [(0, SEGS[1][0]), (SEGS[1][0], SEGS[2][0]),
                   (SEGS[2][0], FG)]
            bounce7 = [dram.tile([NCORES, KL, hi - lo], bf,
                                 name=f"bounce7_{i}")
                       for i, (lo, hi) in enumerate(S7R)]
            recv7 = [dram.tile([NCORES, KL, hi - lo], bf,
                               name=f"recv7_{i}")
                     for i, (lo, hi) in enumerate(S7R)]
            y1snd1 = dram.tile([H1, 32], dt)
            y1rcv1 = dram.tile([H1, 32], dt)
            y1snd2 = dram.tile([H1, 32], dt)
            y1rcv2 = dram.tile([H1, 32], dt)

            with (
                tc.tile_pool(name="cpsum", bufs=2, space="PSUM") as cpsum,
                tc.tile_pool(name="rpsum", bufs=2, space="PSUM") as rpsum,
                tc.tile_pool(name="fpsum", bufs=1, space="PSUM") as fpsum,
            ):
                def build_table(tab, hs, f0, w):
                    """REP-matmul an h chunk [K, w] into the packed table."""
                    tb = tab[:].bitcast(bf).rearrange(
                        "p (f two) -> p f two", two=2)
                    pse = rpsum.tile([128, CHUNK], dt, tag="rp")
                    nc.tensor.matmul(out=pse[:, :w], lhsT=repe_t[:],
                                     rhs=hs[:, :w], start=True, stop=True)
                    nc.vector.tensor_copy(tb[:, f0:f0 + w, 0], pse[:, :w])
                    pso = rpsum.tile([128, CHUNK], dt, tag="rp")
                    nc.tensor.matmul(out=pso[:, :w], lhsT=repo_t[:],
                                     rhs=hs[:, :w], start=True, stop=True)
                    nc.scalar.copy(tb[:, f0:f0 + w, 1], pso[:, :w])

                st = {}  # per-sample tile state

                def load_idx(s):
                    idx_t = idxp.tile([128, WCOL], i16, tag="it")
                    nc.sync.dma_start(idx_t[:], idx16[s])
                    st[s] = dict(idx=idx_t)

                XBLK = 2252  # x staged in 4 big DMAs instead of 18 small

                def conv1_full(s, tab1):
                    st[s]["tab1"] = tab1
                    for xb0, xbw in _chunks(0, FG, XBLK):
                        xc = xbp.tile([CN, XBLK], bf, tag="xb")
                        nc.sync.dma_start(xc[:, :xbw],
                                          xt[s, :, xb0:xb0 + xbw])
                        for f0, w in _chunks(xb0, xbw, CHUNK):
                            lo = f0 - xb0
                            ps = cpsum.tile([K, CHUNK], dt, tag="cp")
                            nc.tensor.matmul(out=ps[:, :w], lhsT=w1_t[:],
                                             rhs=xc[:, lo:lo + w],
                                             start=True, stop=True)
                            hs = hstp.tile([K, CHUNK], bf, tag="hst")
                            nc.vector.tensor_copy(hs[:, :w], ps[:, :w])
                            build_table(tab1, hs, f0, w)

                def gather_seg(s, tab_key, seg):
                    s0, slen = SEGS[seg]
                    go = gop.tile([128, SEGMAX], u32, tag="go")
                    nc.gpsimd.ap_gather(
                        out_ap=go[:, :slen], in_ap=st[s][tab_key][:],
                        idxs_ap=st[s]["idx"][:, s0 // 16:(s0 + slen) // 16],
                        channels=128, num_elems=FG, d=1, num_idxs=slen)
                    return go

                def stage_tail(gos):
                    """SBUF->SBUF DMA group-7's gathered entries into the
                    main (n, kp) partition layout: tailbuf[16n+kp, u] =
                    go[112+kp, n*TL+u]. Returns the [CHT, TL] u32 tile."""
                    tb = tbp.tile([CHT, TL], u32, tag="tb")
                    for n, si, lo, rw, u0 in tail_pieces():
                        nc.sync.dma_start(
                            tb[16 * n:16 * n + 16, u0:u0 + rw],
                            gos[si][112:128, lo:lo + rw])
                    return tb

                def conv_segs(gos, we_t, wo_t, sink, seg_ids):
                    """Conv faces covered by the given gather segments."""
                    for si in seg_ids:
                        s0, slen = SEGS[si]
                        gb = gos[si][:, :slen].bitcast(bf).rearrange(
                            "p (f two) -> p f two", two=2)
                        for f0, w in _chunks(s0, slen, CHUNK):
                            lo = f0 - s0
                            ps = cpsum.tile([K, CHUNK], dt, tag="cp")
                            nc.tensor.matmul(out=ps[:, :w],
                                             lhsT=we_t[:],
                                             rhs=gb[0:CHT, lo:lo + w, 0],
                                             start=True, stop=False)
                            nc.tensor.matmul(out=ps[:, :w],
                                             lhsT=wo_t[:],
                                             rhs=gb[0:CHT, lo:lo + w, 1],
                                             start=False, stop=True)
                            sink(f0, w, ps)

                def conv_tail(gos, we_t, wo_t, sink):
                    """Conv the tail faces [FL, FG) via the restaged
                    group-7 entries."""
                    if FL >= FG:
                        return
                    tb = stage_tail(gos)
                    tbb = tb[:].bitcast(bf).rearrange(
                        "p (f two) -> p f two", two=2)
                    for f0, w in _chunks(FL, FG - FL, CHUNK):
                        lo = f0 - FL
                        ps = cpsum.tile([K, CHUNK], dt, tag="cp")
                        nc.tensor.matmul(out=ps[:, :w], lhsT=we_t[:],
                                         rhs=tbb[:, lo:lo + w, 0],
                                         start=True, stop=False)
                        nc.tensor.matmul(out=ps[:, :w], lhsT=wo_t[:],
                                         rhs=tbb[:, lo:lo + w, 1],
                                         start=False, stop=True)
                        sink(f0, w, ps)

                def conv2_full(s, gos, tab2):
                    st[s]["tab2"] = tab2

                    def sink(f0, w, ps):
                        hs = hstp.tile([K, CHUNK], bf, tag="hst")
                        nc.vector.tensor_copy(hs[:, :w], ps[:, :w])
                        build_table(tab2, hs, f0, w)
                    conv_segs(gos, w2e_t, w2o_t, sink, [0, 1, 2])
                    conv_tail(gos, w2e_t, w2o_t, sink)

                def conv3_sink(s):
                    hs3 = hp.tile([K, FG], bf, tag="h3", name=f"h3_{s}")

                    def sink(f0, w, ps):
                        nc.vector.tensor_copy(hs3[:, f0:f0 + w], ps[:, :w])
                    return hs3, sink

                def conv3_full(s, gos):
                    hs3, sink = conv3_sink(s)
                    conv_segs(gos, w3e_t, w3o_t, sink, [0, 1, 2])
                    conv_tail(gos, w3e_t, w3o_t, sink)
                    # single bounce write per sample: the AllToAll's input
                    # must have one writer (chunked writers race the
                    # collective on HW).
                    nc.sync.dma_start(bounce[s], hs3[:])

                def cc_sample(s):
                    nc.gpsimd.collective_compute(
                        "AllToAll", mybir.AluOpType.bypass,
                        replica_groups=rg,
                        ins=[bounce[s].opt()], outs=[recv[s].opt()])

                # ---- fc1: y1ps[:, cols] += fc1wt.T @ transposed recv rows,
                # accumulated per column group over KL x 71 PSUM chunks.
                y1ps = fpsum.tile([H1, B], dt, tag="y1")
                fc1_state = {}

                def fc1_part(grp, c0, ncols, bblocks, load_rows, total_nst):
                    stt = fc1_state.setdefault(grp, dict(stp=0))
                    for kl in range(KL):
                        for b0, bw in bblocks:
                            lt_in = work.tile([ncols, BLK], bf,
                                              tag=f"ltin{ncols}")
                            load_rows(lt_in, kl, b0, bw)
                            r0 = kl * FG + b0
                            nfull = bw // 128
                            wt = work.tile([128, (BLK // 128) * H1], bf,
                                           tag="fw")
                            if nfull:
                                nc.sync.dma_start(
                                    wt[:, :nfull * H1].rearrange(
                                        "p (c h) -> p c h", h=H1),
                                    fc1wt[r0:r0 + nfull * 128, :].rearrange(
                                        "(c p) h -> p c h", p=128))
                            for ci, (s0c, wc) in enumerate(
                                    _chunks(0, bw, 128)):
                                pst = rpsum.tile([128, 16], bf, tag="tT")
                                nc.tensor.transpose(
                                    pst[:wc, :ncols],
                                    lt_in[:, s0c:s0c + wc],
                                    identB[:ncols, :ncols])
                                ltt = work.tile([128, ncols], bf,
                                                tag=f"ltt{ncols}")
                                nc.vector.tensor_copy(ltt[:wc, :],
                                                      pst[:wc, :ncols])
                                if wc == 128:
                                    lhsT = wt[:, ci * H1:(ci + 1) * H1]
                                else:
                                    wtp = work.tile([128, H1], bf, tag="fwp")
                                    nc.sync.dma_start(
                                        wtp[:wc, :],
                                        fc1wt[r0 + s0c:r0 + s0c + wc, :])
                                    lhsT = wtp[:wc, :]
                                nc.tensor.matmul(
                                    out=y1ps[:, c0:c0 + ncols],
                                    lhsT=lhsT, rhs=ltt[:wc, :],
                                    start=(stt["stp"] == 0),
                                    stop=(stt["stp"] == total_nst - 1))
                                stt["stp"] += 1

                FULL_BLOCKS = _chunks(0, FG, BLK)
                NST_FULL = KL * sum(len(_chunks(0, bw, 128))
                                    for _, bw in FULL_BLOCKS)

                def fc1_half(h):
                    def load_rows(t, kl, b0, bw):
                        nc.sync.dma_start(
                            t[:, :bw], recv[2 * h:2 * h + 2, :, kl,
                                            b0:b0 + bw])
                    fc1_part(f"h{h}", 16 * h, 16, FULL_BLOCKS, load_rows,
                             NST_FULL)

                BLOCKS_7 = [_chunks(lo, hi - lo, BLK) for lo, hi in S7R]
                NST_P7 = KL * sum(len(_chunks(0, bw, 128))
                                  for blocks in BLOCKS_7
                                  for _, bw in blocks)

                def fc1_half3(part):
                    """cols 48:64 = samples 6,7; face-range `part` so each
                    part unblocks as soon as its sample-7 AllToAll lands."""
                    src, off = recv7[part], S7R[part][0]

                    def load_rows(t, kl, b0, bw):
                        nc.sync.dma_start(
                            t[0:8, :bw], recv[6:7, :, kl, b0:b0 + bw])
                        nc.sync.dma_start(
                            t[8:16, :bw],
                            src[:, kl, b0 - off:b0 - off + bw])
                    fc1_part("h3", 48, 16, BLOCKS_7[part], load_rows, NST_P7)

                def new_tab(kind, s):
                    return tabsp.tile([128, FG], u32, tag="tab",
                                      name=f"tab{kind}_{s}")

                def a2a(src, dst):
                    nc.gpsimd.collective_compute(
                        "AllToAll", mybir.AluOpType.bypass,
                        replica_groups=rg,
                        ins=[src[:].opt()], outs=[dst[:].opt()])

                def allreduce(src, dst):
                    nc.gpsimd.collective_compute(
                        "AllReduce", mybir.AluOpType.add, replica_groups=rg,
                        ins=[src[:].opt()], outs=[dst[:].opt()])

                # ---- prologue: sample-0 table + first gathers before all
                # other constants, so the head is just idx0+conv1(0).
                load_idx(0)
                # keep the (otherwise unused) xcp pool's footprint so the
                # SBUF layout of the pools behind it doesn't shift — the
                # ap_gather ucode rate is sensitive to table placement.
                xpad0 = xcp.tile([CN, CHUNK], bf, tag="xc")
                nc.vector.memset(xpad0[:, :8], 0.0)
                xpad1 = xcp.tile([CN, CHUNK], bf, tag="xc")
                nc.vector.memset(xpad1[:, :8], 0.0)
                conv1_full(0, new_tab(1, 0))
                gos0 = [gather_seg(0, "tab1", i) for i in range(3)]
                load_idx(1)
                conv1_full(1, new_tab(1, 1))

                w2e_t = consts.tile([CHT, K], bf)
                nc.sync.dma_start(w2e_t[:], w2e[:])
                w2o_t = consts.tile([CHT, K], bf)
                nc.sync.dma_start(w2o_t[:], w2o[:])
                w3e_t = consts.tile([CHT, K], bf)
                nc.sync.dma_start(w3e_t[:], w3e[:])
                w3o_t = consts.tile([CHT, K], bf)
                nc.sync.dma_start(w3o_t[:], w3o[:])
                identB = consts.tile([B, B], bf)
                make_identity(nc, identB)
                zcol = consts.tile([128, 1], dt)
                nc.vector.memset(zcol[:], 0.0)

                conv2_full(0, gos0, new_tab(2, 0))

                # ---- software-pipelined sample loop ----
                for s in range(BL):
                    nxt = s + 1 < BL
                    # table slot rotation in conv1-then-conv2 order (the
                    # 3-slot cycle then always lands writers on slots whose
                    # readers finished an iteration ago)
                    t1n = new_tab(1, s + 2) if s + 2 < BL else None
                    t2n = new_tab(2, s + 1) if nxt else None
                    if nxt:
                        gos2 = [gather_seg(s + 1, "tab1", i)
                                for i in range(3)]
                    if s >= 1:
                        cc_sample(s - 1)
                    if s < BL - 1:
                        gos3 = [gather_seg(s, "tab2", i) for i in range(3)]
                        if s + 2 < BL:
                            load_idx(s + 2)
                            conv1_full(s + 2, t1n)
                        if nxt:
                            conv2_full(s + 1, gos2, t2n)
                        conv3_full(s, gos3)
                        if s == 3:
                            fc1_half(0)
                        elif s == 5:
                            fc1_half(1)
                        elif s == 6:
                            # AllReduce of halves 0,1 (cols 0:32): fired
                            # inside the loop, off the tail critical path
                            y1l1 = work.tile([H1, 32], dt, tag="y1l1")
                            nc.vector.tensor_copy(y1l1[:], y1ps[:, 0:32])
                            nc.sync.dma_start(y1snd1[:], y1l1[:])
                            allreduce(y1snd1, y1rcv1)
                    else:
                        # ---- last sample: half2 fills iter-7's PE slack
                        # (emitted before the gathers so its waits don't get
                        # sem-merged with them); the AllToAll is split in 3
                        # face ranges so the exchange and fc1 half3 overlap
                        # the final gathers.
                        fc1_half(2)
                        g3a = gather_seg(s, "tab2", 0)
                        g3b = gather_seg(s, "tab2", 1)
                        hs3, sink = conv3_sink(s)
                        conv_segs([g3a], w3e_t, w3o_t, sink, [0])
                        nc.sync.dma_start(bounce7[0][:],
                                          hs3[:, S7R[0][0]:S7R[0][1]])
                        g3c = gather_seg(s, "tab2", 2)
                        gos3 = [g3a, g3b, g3c]
                        # triggers sit after all gathers in the Pool queue:
                        # the sequencer runs ahead of the gather engine, so
                        # each fires as soon as its bounce DMA lands.
                        a2a(bounce7[0], recv7[0])
                        fc1_half3(0)
                        conv_segs(gos3, w3e_t, w3o_t, sink, [1])
                        nc.sync.dma_start(bounce7[1][:],
                                          hs3[:, S7R[1][0]:S7R[1][1]])
                        a2a(bounce7[1], recv7[1])
                        fc1_half3(1)
                        conv_segs(gos3, w3e_t, w3o_t, sink, [2])
                        conv_tail(gos3, w3e_t, w3o_t, sink)
                        nc.sync.dma_start(bounce7[2][:],
                                          hs3[:, S7R[2][0]:S7R[2][1]])
                        a2a(bounce7[2], recv7[2])
                        fc1_half3(2)
                        y1l2 = work.tile([H1, 32], dt, tag="y1l2")
                        nc.vector.tensor_copy(y1l2[:], y1ps[:, 32:64])
                        nc.sync.dma_start(y1snd2[:], y1l2[:])
                        allreduce(y1snd2, y1rcv2)
                    st.pop(s)

                # ---- head (replicated) ----
                def bn_relu(y, h, g_ap, b_ap, relu=True):
                    """In-place batchnorm(+relu) on SBUF tile y [h, B]."""
                    mean = work.tile([h, 1], dt, tag=f"bn_m{h}")
                    nc.vector.reduce_sum(mean[:], y[:],
                                         axis=mybir.AxisListType.X)
                    nc.vector.tensor_scalar_mul(mean[:], mean[:], 1.0 / B)
                    sq = work.tile([h, B], dt, tag=f"bn_sq{h}")
                    nc.vector.tensor_tensor(out=sq[:], in0=y[:], in1=y[:],
                                            op=mybir.AluOpType.mult)
                    var = work.tile([h, 1], dt, tag=f"bn_v{h}")
                    nc.vector.reduce_sum(var[:], sq[:],
                                         axis=mybir.AxisListType.X)
                    nc.vector.tensor_scalar_mul(var[:], var[:], 1.0 / B)
                    m2 = work.tile([h, 1], dt, tag=f"bn_m2{h}")
                    nc.vector.tensor_tensor(out=m2[:], in0=mean[:],
                                            in1=mean[:],
                                            op=mybir.AluOpType.mult)
                    nc.vector.tensor_tensor(out=var[:], in0=var[:], in1=m2[:],
                                            op=mybir.AluOpType.subtract)
                    nc.vector.tensor_scalar_add(var[:], var[:], cfg.EPS)
                    std = work.tile([h, 1], dt, tag=f"bn_s{h}")
                    nc.scalar.activation(std[:], var[:],
                                         mybir.ActivationFunctionType.Sqrt,
                                         bias=zcol[:h, :1])
                    rstd = work.tile([h, 1], dt, tag=f"bn_r{h}")
                    nc.vector.reciprocal(rstd[:], std[:])
                    gl = work.tile([h, 1], dt, tag=f"bn_g{h}")
                    nc.sync.dma_start(gl[:], g_ap[:])
                    bl = work.tile([h, 1], dt, tag=f"bn_b{h}")
                    nc.sync.dma_start(bl[:], b_ap[:])
                    scale = work.tile([h, 1], dt, tag=f"bn_sc{h}")
                    nc.vector.tensor_tensor(out=scale[:], in0=rstd[:],
                                            in1=gl[:],
                                            op=mybir.AluOpType.mult)
                    shift = work.tile([h, 1], dt, tag=f"bn_sh{h}")
                    nc.vector.tensor_tensor(out=shift[:], in0=mean[:],
                                            in1=scale[:],
                                            op=mybir.AluOpType.mult)
                    nc.vector.tensor_tensor(out=shift[:], in0=bl[:],
                                            in1=shift[:],
                                            op=mybir.AluOpType.subtract)
                    nc.vector.tensor_scalar(
                        out=y[:], in0=y[:], scalar1=scale[:], scalar2=shift[:],
                        op0=mybir.AluOpType.mult, op1=mybir.AluOpType.add)
                    if relu:
                        nc.scalar.activation(y[:], y[:],
                                             mybir.ActivationFunctionType.Relu,
                                             bias=zcol[:h, :1])

                y1 = work.tile([H1, B], dt, tag="y1h")
                nc.sync.dma_start(y1[:, 0:32], y1rcv1[:])
                nc.sync.dma_start(y1[:, 32:64], y1rcv2[:])
                f1b = work.tile([H1, 1], dt, tag="f1b")
                nc.sync.dma_start(f1b[:], fc1b[:])
                nc.vector.tensor_scalar_add(y1[:], y1[:], f1b[:])
                bn_relu(y1, H1, bn1g, bn1b)

                w2f = work.tile([H1, H2], dt, tag="w2f")
                nc.sync.dma_start(w2f[:], fc2wt[:])
                ps2 = cpsum.tile([K, CHUNK], dt, tag="cp")
                nc.tensor.matmul(out=ps2[0:H2, 0:B], lhsT=w2f[:], rhs=y1[:],
                                 start=True, stop=True)
                y2 = work.tile([H2, B], dt, tag="y2h")
                nc.vector.tensor_copy(y2[:], ps2[0:H2, 0:B])
                f2b = work.tile([H2, 1], dt, tag="f2b")
                nc.sync.dma_start(f2b[:], fc2b[:])
                nc.vector.tensor_scalar_add(y2[:], y2[:], f2b[:])
                bn_relu(y2, H2, bn2g, bn2b)

                wof = work.tile([H2, NCLS], dt, tag="wof")
                nc.sync.dma_start(wof[:], fcowt[:])
                pso = cpsum.tile([K, CHUNK], dt, tag="cp")
                nc.tensor.matmul(out=pso[0:NCLS, 0:B], lhsT=wof[:], rhs=y2[:],
                                 start=True, stop=True)
                yo = work.tile([NCLS, B], dt, tag="yo")
                nc.vector.tensor_copy(yo[:], pso[0:NCLS, 0:B])
                fob = work.tile([NCLS, 1], dt, tag="fob")
                nc.sync.dma_start(fob[:], fcob[:])
                nc.vector.tensor_scalar_add(yo[:], yo[:], fob[:])
                nc.sync.dma_start(out[:], yo[:])

    nc.compile()
    return nc


_CACHE: dict = {}


def _get_program(cfg: Cfg):
    key = cfg
    if key not in _CACHE:
        _CACHE[key] = build_program(cfg)
    return _CACHE[key]


def kernel(**inputs) -> np.ndarray:
    from concourse import bass_utils

    cfg = CFG
    nc = _get_program(cfg)
    in_maps = prep_core_inputs(cfg, **inputs)
    res = bass_utils.run_bass_kernel_spmd(
        nc, in_maps, core_ids=list(range(cfg.ncores)))
    return postprocess(res.results[0]["out"], cfg)


# revision 22
# speedup vs baseline: 1.2124x; 1.0136x over previous
"""Trainium2 Bass kernel for nn_CNN_9818295238933 (gnn_message_passing).

Data-parallel over batch across 8 cores (8 samples each). Per sample:
  conv1 (PE, bf16) -> h1 [32, F] -> REP matmul replicates h1 across 8
  partition groups as a bf16-pair-packed SBUF table [128, F] (partition
  (g, kp) holds the bf16 pair (h[2kp], h[2kp+1]) at face f).
  ap_gather (GPSIMD, SBUF-local) gathers the table with that sample's
  adjacency: groups 0-6 carry neighbour slot n for faces [0, FL); group 7
  carries the tail faces [FL, FG) of all 7 slots concatenated, so all 8
  Q7 cores work and each instruction processes FL/seg indices instead of
  FG. Gathered tiles feed the next conv directly as strided bf16 matmul
  rhs (contraction over (n, kp) partitions, even/odd k accumulated in
  PSUM); tail faces get per-n 16-partition matmuls from group 7's slice.
  Repeat for conv2 -> table2 -> gather -> conv3.

The ap_gather ucode is the hard bottleneck (~27.3 ns/index per Q7 core,
measured on idle HW; 48 segment-gathers x ~71.3 us = 3.42 ms). The
sample loop is software-pipelined so the Pool engine never waits:
gathers are emitted as [g2(s+1) segs][cc(s-1)][g3(s) segs]; convs are
emitted consumer-first (conv2(s+1), conv3(s) BEFORE conv1(s+2)) so PE
frees gather buffers promptly. All three tables (tab1 x2 live +
tab2 x1) share one tag-rotated 3-slot pool, which double-buffers tab1
at startup (kills the prologue stall) while keeping baseline SBUF use.

Head: idx(0)+conv1(0) emitted before all other constants. Tail: h3
bounces to DRAM per sample and a per-sample AllToAll redistributes
k-slices; fc1 runs as 3 16-col halves (iters 3/5/6) + sample-6 and
sample-7 8-col pieces; sample 7's AllToAll is split at face 2624 so
its first third (and fc1 piece7A) overlaps the final gather segment;
AllReduce is split in 3 column groups (2 fire during the loop).
BN+ReLU+fc2+BN+ReLU+fco replicated.

Self-contained: hardcodes all shapes; only imports the Trainium toolchain.
"""

import sys
from dataclasses import dataclass

if "/opt/trn_rl_repo" not in sys.path:
    sys.path.insert(0, "/opt/trn_rl_repo")

import numpy as np


@dataclass(frozen=True)
class Cfg:
    ncores: int = 8
    B: int = 64
    C: int = 12
    N: int = 7
    K: int = 32
    F: int = 9000
    FG: int = 9008          # compute/table extent (F padded to mult of 16)
    FL: int = 7888          # main faces per neighbour group (= FG * 7/8 pad16)
    H1: int = 100
    H2: int = 30
    NCLS: int = 2
    EPS: float = 1e-5
    CHUNK: int = 512        # PSUM f-chunk

    @property
    def BL(self):
        return self.B // self.ncores

    @property
    def CN(self):
        return self.C * self.N

    @property
    def KL(self):
        return self.K // self.ncores

    @property
    def KP(self):
        return self.K // 2

    @property
    def CHT(self):
        return self.N * self.KP  # 112 main channels

    @property
    def TL(self):
        return self.FG - self.FL  # 1120 tail faces

    @property
    def SEGS(self):
        # Segment starts must be multiples of 32 entries: the gather
        # ucode reads the wrapped idx list as u32 words, and a 2-byte
        # misaligned base corrupts words crossing 16-byte boundaries.
        if self.FL == 9008:  # tail disabled
            return [(0, 3008), (3008, 3008), (6016, 2992)]
        return [(0, 2624), (2624, 2624), (5248, 2640)]

    @property
    def WCOL(self):
        return self.FL // 16  # wrapped idx columns (493)

    @property
    def SPLIT(self):
        return self.SEGS[1][0]  # face split for sample-7's early AllToAll


CFG = Cfg()


def _chunks(f0, flen, step):
    out = []
    f = f0
    while f < f0 + flen:
        out.append((f, min(step, f0 + flen - f)))
        f += step
    return out


# ---------------------------------------------------------------------------
# Host-side input preparation
# ---------------------------------------------------------------------------

def prep_core_inputs(cfg: Cfg, x, adjacencies, W1, W2, W3, fc1_w, fc1_b, bn1_g,
                     bn1_b, fc2_w, fc2_b, bn2_g, bn2_b, fco_w, fco_b):
    import ml_dtypes
    bf16 = ml_dtypes.bfloat16

    B, C, N, K, F, FG, FL = (cfg.B, cfg.C, cfg.N, cfg.K, cfg.F, cfg.FG,
                             cfg.FL)
    BL, CN, KL, KP, TL = cfg.BL, cfg.CN, cfg.KL, cfg.KP, cfg.TL
    H1, H2, NCLS = cfg.H1, cfg.H2, cfg.NCLS

    x = np.asarray(x, dtype=np.float32)
    adj = np.asarray(adjacencies).astype(np.int64)[:, 0]  # [B, F, N]

    # x [B, C, F, N] -> xt [B, (c,n), FG] bf16, zero-padded along f.
    xt = np.zeros((B, CN, FG), dtype=bf16)
    xt[:, :, :F] = np.transpose(x, (0, 1, 3, 2)).reshape(B, CN, F).astype(bf16)

    # Gather index lists, one per 16-partition group:
    #   group n < 7: adj[b, f, n] for f in [0, FL)
    #   group 7:     adj[b, FL+u, n] at position n*TL+u (pad to FL with 0)
    # wrapped so entry i sits at [16g + i%16, i//16]. Segment boundaries
    # are multiples of 16 so column-slicing yields each segment's list.
    idx_pad = np.zeros((B, FG, N), dtype=np.int64)
    idx_pad[:, :F] = adj
    lists = np.zeros((B, 8, FL), dtype=np.int64)
    lists[:, :7, :] = np.transpose(idx_pad[:, :FL], (0, 2, 1))
    lists[:, 7, :N * TL] = np.transpose(
        idx_pad[:, FL:], (0, 2, 1)).reshape(B, N * TL)
    wrap = lists.reshape(B, 8, FL // 16, 16)
    idx16 = np.ascontiguousarray(
        np.transpose(wrap, (0, 1, 3, 2)).reshape(B, 128, FL // 16)
    ).astype(np.int16)

    w1f = np.transpose(np.asarray(W1, np.float32), (1, 2, 0)).reshape(CN, K)

    def eo(Wm):  # [K_out, K_in, N] -> even/odd lhsT [(n,kp), K_out] bf16
        Wm = np.asarray(Wm, np.float32)
        we = np.transpose(Wm[:, 0::2, :], (2, 1, 0)).reshape(N * KP, K)
        wo = np.transpose(Wm[:, 1::2, :], (2, 1, 0)).reshape(N * KP, K)
        return (np.ascontiguousarray(we).astype(bf16),
                np.ascontiguousarray(wo).astype(bf16))

    w2e, w2o = eo(W2)
    w3e, w3o = eo(W3)

    # Replication matrices over all 8 groups: repe[q, (g,kp)] = (q == 2*kp)
    q = np.arange(K)[:, None]
    p = np.arange(128)[None, :]
    repe = (q == 2 * (p % KP)).astype(bf16)
    repo = (q == 2 * (p % KP) + 1).astype(bf16)

    # fc1 weights: [H1, K*F] -> [K, FG, H1] zero-padded, per-core k-slice.
    fc1 = np.asarray(fc1_w, np.float32).reshape(H1, K, F)
    fc1t = np.zeros((K, FG, H1), dtype=bf16)
    fc1t[:, :F] = np.transpose(fc1, (1, 2, 0)).astype(bf16)

    fc2wt = np.ascontiguousarray(np.asarray(fc2_w, np.float32).T)  # [H1, H2]
    fcowt = np.ascontiguousarray(np.asarray(fco_w, np.float32).T)  # [H2, NCLS]

    def col(v, n):
        return np.asarray(v, np.float32).reshape(n, 1)

    shared = dict(
        w1=w1f.astype(bf16), w2e=w2e, w2o=w2o, w3e=w3e, w3o=w3o,
        repe=repe, repo=repo,
        fc1b=col(fc1_b, H1), bn1g=col(bn1_g, H1), bn1b=col(bn1_b, H1),
        fc2wt=fc2wt, fc2b=col(fc2_b, H2), bn2g=col(bn2_g, H2),
        bn2b=col(bn2_b, H2), fcowt=fcowt, fcob=col(fco_b, NCLS),
    )

    in_maps = []
    for c in range(cfg.ncores):
        bsl = slice(c * BL, (c + 1) * BL)
        fc1wt_c = np.ascontiguousarray(
            fc1t[c * KL:(c + 1) * KL].reshape(KL * FG, H1))
        m = dict(shared)
        m.update(
            xt=np.ascontiguousarray(xt[bsl]),
            idx16=np.ascontiguousarray(idx16[bsl]),
            fc1wt=fc1wt_c,
        )
        in_maps.append(m)
    return in_maps


def postprocess(out_dev: np.ndarray, cfg: Cfg = CFG) -> np.ndarray:
    """Device out columns are (sample-within-core, core) ordered; return
    [B, NCLS] in global sample order (core-major)."""
    o = np.asarray(out_dev, np.float32).reshape(cfg.NCLS, cfg.BL, cfg.ncores)
    return np.ascontiguousarray(o.transpose(2, 1, 0).reshape(cfg.B, cfg.NCLS))


# ---------------------------------------------------------------------------
# Device program
# ---------------------------------------------------------------------------

def build_program(cfg: Cfg):
    import concourse.bass as bass  # noqa: F401
    import concourse.bacc as bacc
    import concourse.mybir as mybir
    import concourse.tile as tile
    from concourse.masks import make_identity

    dt = mybir.dt.float32
    bf = mybir.dt.bfloat16
    u32 = mybir.dt.uint32
    i16 = mybir.dt.int16
    B, C, N, K, FG, FL = cfg.B, cfg.C, cfg.N, cfg.K, cfg.FG, cfg.FL
    BL, CN, KL, KP, CHT, TL = (cfg.BL, cfg.CN, cfg.KL, cfg.KP, cfg.CHT,
                               cfg.TL)
    H1, H2, NCLS = cfg.H1, cfg.H2, cfg.NCLS
    CHUNK, SEGS, WCOL = cfg.CHUNK, cfg.SEGS, cfg.WCOL
    SPLIT = cfg.SPLIT
    NCORES = cfg.ncores
    SEGMAX = max(w for _, w in SEGS)
    BLK = 1024
    rg = [list(range(NCORES))]

    nc = bacc.Bacc("TRN2", target_bir_lowering=False, debug=False,
                   num_devices=NCORES, num_swdge_queues=4)

    xt = nc.dram_tensor("xt", [BL, CN, FG], bf, kind="ExternalInput")
    idx16 = nc.dram_tensor("idx16", [BL, 128, WCOL], i16,
                           kind="ExternalInput")
    w1 = nc.dram_tensor("w1", [CN, K], bf, kind="ExternalInput")
    w2e = nc.dram_tensor("w2e", [CHT, K], bf, kind="ExternalInput")
    w2o = nc.dram_tensor("w2o", [CHT, K], bf, kind="ExternalInput")
    w3e = nc.dram_tensor("w3e", [CHT, K], bf, kind="ExternalInput")
    w3o = nc.dram_tensor("w3o", [CHT, K], bf, kind="ExternalInput")
    repe = nc.dram_tensor("repe", [K, 128], bf, kind="ExternalInput")
    repo = nc.dram_tensor("repo", [K, 128], bf, kind="ExternalInput")
    fc1wt = nc.dram_tensor("fc1wt", [KL * FG, H1], bf, kind="ExternalInput")
    fc1b = nc.dram_tensor("fc1b", [H1, 1], dt, kind="ExternalInput")
    bn1g = nc.dram_tensor("bn1g", [H1, 1], dt, kind="ExternalInput")
    bn1b = nc.dram_tensor("bn1b", [H1, 1], dt, kind="ExternalInput")
    fc2wt = nc.dram_tensor("fc2wt", [H1, H2], dt, kind="ExternalInput")
    fc2b = nc.dram_tensor("fc2b", [H2, 1], dt, kind="ExternalInput")
    bn2g = nc.dram_tensor("bn2g", [H2, 1], dt, kind="ExternalInput")
    bn2b = nc.dram_tensor("bn2b", [H2, 1], dt, kind="ExternalInput")
    fcowt = nc.dram_tensor("fcowt", [H2, NCLS], dt, kind="ExternalInput")
    fcob = nc.dram_tensor("fcob", [NCLS, 1], dt, kind="ExternalInput")
    out = nc.dram_tensor("out", [NCLS, B], dt, kind="ExternalOutput")

    def tail_pieces():
        """(n, seg_idx, seg_local_start, width, tail_local_start) pieces
        covering each neighbour's [n*TL, (n+1)*TL) slice of group-7's
        entry list, split at gather-segment boundaries."""
        out_runs = []
        for n in range(N):
            e0, e1 = n * TL, (n + 1) * TL
            for si, (s0, slen) in enumerate(SEGS):
                lo = max(e0, s0)
                hi = min(e1, s0 + slen)
                if lo < hi:
                    out_runs.append((n, si, lo - s0, hi - lo, lo - e0))
        return out_runs

    with tile.TileContext(nc) as tc:
        with (
            tc.tile_pool(name="consts", bufs=1) as consts,
            tc.tile_pool(name="xcp", bufs=2) as xcp,
            tc.tile_pool(name="idxp", bufs=3) as idxp,
            tc.tile_pool(name="tabs", bufs=3) as tabsp,
            tc.tile_pool(name="gop", bufs=4) as gop,
            tc.tile_pool(name="tbp", bufs=1) as tbp,
            tc.tile_pool(name="hp", bufs=1) as hp,
            tc.tile_pool(name="hst", bufs=2) as hstp,
            tc.tile_pool(name="work", bufs=2) as work,
            tc.tile_pool(name="xbp", bufs=2) as xbp,
            tc.tile_pool(name="dram", bufs=1, space="DRAM") as dram,
        ):
            # ---- constants needed by conv1(0) go first; the rest are
            # emitted after the first gathers so they don't delay the head.
            w1_t = consts.tile([CN, K], bf)
            nc.sync.dma_start(w1_t[:], w1[:])
            repe_t = consts.tile([K, 128], bf)
            nc.sync.dma_start(repe_t[:], repe[:])
            repo_t = consts.tile([K, 128], bf)
            nc.sync.dma_start(repo_t[:], repo[:])

            bounce = dram.tile([BL, NCORES, KL, FG], bf)
            recv = dram.tile([BL, NCORES, KL, FG], bf)
            # sample-7 AllToAll face-range splits (seg0 / seg1 / seg2+tail)
            S7R = [(0, SEGS[1][0]), (SEGS[1][0], SEGS[2][0]),
                   (SEGS[2][0], FG)]
            bounce7 = [dram.tile([NCORES, KL, hi - lo], bf,
                                 name=f"bounce7_{i}")
                       for i, (lo, hi) in enumerate(S7R)]
            recv7 = [dram.tile([NCORES, KL, hi - lo], bf,
                               name=f"recv7_{i}")
                     for i, (lo, hi) in enumerate(S7R)]
            y1snd1 = dram.tile([H1, 32], dt)
            y1rcv1 = dram.tile([H1, 32], dt)
            y1snd2 = dram.tile([H1, 32], dt)
            y1rcv2 = dram.tile([H1, 32], dt)

            with (
                tc.tile_pool(name="cpsum", bufs=2, space="PSUM") as cpsum,
                tc.tile_pool(name="rpsum", bufs=2, space="PSUM") as rpsum,
                tc.tile_pool(name="fpsum", bufs=1, space="PSUM") as fpsum,
            ):
                def build_table(tab, hs, f0, w):
                    """REP-matmul an h chunk [K, w] into the packed table."""
                    tb = tab[:].bitcast(bf).rearrange(
                        "p (f two) -> p f two", two=2)
                    pse = rpsum.tile([128, CHUNK], dt, tag="rp")
                    nc.tensor.matmul(out=pse[:, :w], lhsT=repe_t[:],
                                     rhs=hs[:, :w], start=True, stop=True)
                    nc.vector.tensor_copy(tb[:, f0:f0 + w, 0], pse[:, :w])
                    pso = rpsum.tile([128, CHUNK], dt, tag="rp")
                    nc.tensor.matmul(out=pso[:, :w], lhsT=repo_t[:],
                                     rhs=hs[:, :w], start=True, stop=True)
                    nc.scalar.copy(tb[:, f0:f0 + w, 1], pso[:, :w])

                st = {}  # per-sample tile state

                def load_idx(s):
                    idx_t = idxp.tile([128, WCOL], i16, tag="it")
                    nc.sync.dma_start(idx_t[:], idx16[s])
                    st[s] = dict(idx=idx_t)

                XBLK = 2252  # x staged in 4 big DMAs instead of 18 small

                def conv1_full(s, tab1):
                    st[s]["tab1"] = tab1
                    for xb0, xbw in _chunks(0, FG, XBLK):
                        xc = xbp.tile([CN, XBLK], bf, tag="xb")
                        nc.sync.dma_start(xc[:, :xbw],
                                          xt[s, :, xb0:xb0 + xbw])
                        for f0, w in _chunks(xb0, xbw, CHUNK):
                            lo = f0 - xb0
                            ps = cpsum.tile([K, CHUNK], dt, tag="cp")
                            nc.tensor.matmul(out=ps[:, :w], lhsT=w1_t[:],
                                             rhs=xc[:, lo:lo + w],
                                             start=True, stop=True)
                            hs = hstp.tile([K, CHUNK], bf, tag="hst")
                            nc.vector.tensor_copy(hs[:, :w], ps[:, :w])
                            build_table(tab1, hs, f0, w)

                def gather_seg(s, tab_key, seg):
                    s0, slen = SEGS[seg]
                    go = gop.tile([128, SEGMAX], u32, tag="go")
                    nc.gpsimd.ap_gather(
                        out_ap=go[:, :slen], in_ap=st[s][tab_key][:],
                        idxs_ap=st[s]["idx"][:, s0 // 16:(s0 + slen) // 16],
                        channels=128, num_elems=FG, d=1, num_idxs=slen)
                    return go

                def stage_tail(gos):
                    """SBUF->SBUF DMA group-7's gathered entries into the
                    main (n, kp) partition layout: tailbuf[16n+kp, u] =
                    go[112+kp, n*TL+u]. Returns the [CHT, TL] u32 tile."""
                    tb = tbp.tile([CHT, TL], u32, tag="tb")
                    for n, si, lo, rw, u0 in tail_pieces():
                        nc.sync.dma_start(
                            tb[16 * n:16 * n + 16, u0:u0 + rw],
                            gos[si][112:128, lo:lo + rw])
                    return tb

                def conv_segs(gos, we_t, wo_t, sink, seg_ids):
                    """Conv faces covered by the given gather segments."""
                    for si in seg_ids:
                        s0, slen = SEGS[si]
                        gb = gos[si][:, :slen].bitcast(bf).rearrange(
                            "p (f two) -> p f two", two=2)
                        for f0, w in _chunks(s0, slen, CHUNK):
                            lo = f0 - s0
                            ps = cpsum.tile([K, CHUNK], dt, tag="cp")
                            nc.tensor.matmul(out=ps[:, :w],
                                             lhsT=we_t[:],
                                             rhs=gb[0:CHT, lo:lo + w, 0],
                                             start=True, stop=False)
                            nc.tensor.matmul(out=ps[:, :w],
                                             lhsT=wo_t[:],
                                             rhs=gb[0:CHT, lo:lo + w, 1],
                                             start=False, stop=True)
                            sink(f0, w, ps)

                def conv_tail(gos, we_t, wo_t, sink):
                    """Conv the tail faces [FL, FG) via the restaged
                    group-7 entries."""
                    if FL >= FG:
                        return
                    tb = stage_tail(gos)
                    tbb = tb[:].bitcast(bf).rearrange(
                        "p (f two) -> p f two", two=2)
                    for f0, w in _chunks(FL, FG - FL, CHUNK):
                        lo = f0 - FL
                        ps = cpsum.tile([K, CHUNK], dt, tag="cp")
                        nc.tensor.matmul(out=ps[:, :w], lhsT=we_t[:],
                                         rhs=tbb[:, lo:lo + w, 0],
                                         start=True, stop=False)
                        nc.tensor.matmul(out=ps[:, :w], lhsT=wo_t[:],
                                         rhs=tbb[:, lo:lo + w, 1],
                                         start=False, stop=True)
                        sink(f0, w, ps)

                def conv2_full(s, gos, tab2):
                    st[s]["tab2"] = tab2

                    def sink(f0, w, ps):
                        hs = hstp.tile([K, CHUNK], bf, tag="hst")
                        nc.vector.tensor_copy(hs[:, :w], ps[:, :w])
                        build_table(tab2, hs, f0, w)
                    conv_segs(gos, w2e_t, w2o_t, sink, [0, 1, 2])
                    conv_tail(gos, w2e_t, w2o_t, sink)

                def conv3_sink(s):
                    hs3 = hp.tile([K, FG], bf, tag="h3", name=f"h3_{s}")

                    def sink(f0, w, ps):
                        nc.vector.tensor_copy(hs3[:, f0:f0 + w], ps[:, :w])
                    return hs3, sink

                def conv3_full(s, gos):
                    hs3, sink = conv3_sink(s)
                    conv_segs(gos, w3e_t, w3o_t, sink, [0, 1, 2])
                    conv_tail(gos, w3e_t, w3o_t, sink)
                    # single bounce write per sample: the AllToAll's input
                    # must have one writer (chunked writers race the
                    # collective on HW).
                    nc.sync.dma_start(bounce[s], hs3[:])

                def cc_sample(s):
                    nc.gpsimd.collective_compute(
                        "AllToAll", mybir.AluOpType.bypass,
                        replica_groups=rg,
                        ins=[bounce[s].opt()], outs=[recv[s].opt()])

                # ---- fc1: y1ps[:, cols] += fc1wt.T @ transposed recv rows,
                # accumulated per column group over KL x 71 PSUM chunks.
                y1ps = fpsum.tile([H1, B], dt, tag="y1")
                fc1_state = {}

                def fc1_part(grp, c0, ncols, bblocks, load_rows, total_nst):
                    stt = fc1_state.setdefault(grp, dict(stp=0))
                    for kl in range(KL):
                        for b0, bw in bblocks:
                            lt_in = work.tile([ncols, BLK], bf,
                                              tag=f"ltin{ncols}")
                            load_rows(lt_in, kl, b0, bw)
                            r0 = kl * FG + b0
                            nfull = bw // 128
                            wt = work.tile([128, (BLK // 128) * H1], bf,
                                           tag="fw")
                            if nfull:
                                nc.scalar.dma_start(
                                    wt[:, :nfull * H1].rearrange(
                                        "p (c h) -> p c h", h=H1),
                                    fc1wt[r0:r0 + nfull * 128, :].rearrange(
                                        "(c p) h -> p c h", p=128))
                            for ci, (s0c, wc) in enumerate(
                                    _chunks(0, bw, 128)):
                                pst = rpsum.tile([128, 16], bf, tag="tT")
                                nc.tensor.transpose(
                                    pst[:wc, :ncols],
                                    lt_in[:, s0c:s0c + wc],
                                    identB[:ncols, :ncols])
                                ltt = work.tile([128, ncols], bf,
                                                tag=f"ltt{ncols}")
                                nc.vector.tensor_copy(ltt[:wc, :],
                                                      pst[:wc, :ncols])
                                if wc == 128:
                                    lhsT = wt[:, ci * H1:(ci + 1) * H1]
                                else:
                                    wtp = work.tile([128, H1], bf, tag="fwp")
                                    nc.scalar.dma_start(
                                        wtp[:wc, :],
                                        fc1wt[r0 + s0c:r0 + s0c + wc, :])
                                    lhsT = wtp[:wc, :]
                                nc.tensor.matmul(
                                    out=y1ps[:, c0:c0 + ncols],
                                    lhsT=lhsT, rhs=ltt[:wc, :],
                                    start=(stt["stp"] == 0),
                                    stop=(stt["stp"] == total_nst - 1))
                                stt["stp"] += 1

                FULL_BLOCKS = _chunks(0, FG, BLK)
                NST_FULL = KL * sum(len(_chunks(0, bw, 128))
                                    for _, bw in FULL_BLOCKS)

                def fc1_half(h):
                    def load_rows(t, kl, b0, bw):
                        nc.scalar.dma_start(
                            t[:, :bw], recv[2 * h:2 * h + 2, :, kl,
                                            b0:b0 + bw])
                    fc1_part(f"h{h}", 16 * h, 16, FULL_BLOCKS, load_rows,
                             NST_FULL)

                BLOCKS_7 = [_chunks(lo, hi - lo, BLK) for lo, hi in S7R]
                NST_P7 = KL * sum(len(_chunks(0, bw, 128))
                                  for blocks in BLOCKS_7
                                  for _, bw in blocks)

                def fc1_half3(part):
                    """cols 48:64 = samples 6,7; face-range `part` so each
                    part unblocks as soon as its sample-7 AllToAll lands."""
                    src, off = recv7[part], S7R[part][0]

                    def load_rows(t, kl, b0, bw):
                        nc.scalar.dma_start(
                            t[0:8, :bw], recv[6:7, :, kl, b0:b0 + bw])
                        nc.scalar.dma_start(
                            t[8:16, :bw],
                            src[:, kl, b0 - off:b0 - off + bw])
                    fc1_part("h3", 48, 16, BLOCKS_7[part], load_rows, NST_P7)

                def new_tab(kind, s):
                    return tabsp.tile([128, FG], u32, tag="tab",
                                      name=f"tab{kind}_{s}")

                def a2a(src, dst):
                    nc.gpsimd.collective_compute(
                        "AllToAll", mybir.AluOpType.bypass,
                        replica_groups=rg,
                        ins=[src[:].opt()], outs=[dst[:].opt()])

                def allreduce(src, dst):
                    nc.gpsimd.collective_compute(
                        "AllReduce", mybir.AluOpType.add, replica_groups=rg,
                        ins=[src[:].opt()], outs=[dst[:].opt()])

                # ---- prologue: sample-0 table + first gathers before all
                # other constants, so the head is just idx0+conv1(0).
                load_idx(0)
                # keep the (otherwise unused) xcp pool's footprint so the
                # SBUF layout of the pools behind it doesn't shift — the
                # ap_gather ucode rate is sensitive to table placement.
                xpad0 = xcp.tile([CN, CHUNK], bf, tag="xc")
                nc.vector.memset(xpad0[:, :8], 0.0)
                xpad1 = xcp.tile([CN, CHUNK], bf, tag="xc")
                nc.vector.memset(xpad1[:, :8], 0.0)
                conv1_full(0, new_tab(1, 0))
                gos0 = [gather_seg(0, "tab1", i) for i in range(3)]
                load_idx(1)
                conv1_full(1, new_tab(1, 1))

                w2e_t = consts.tile([CHT, K], bf)
                nc.sync.dma_start(w2e_t[:], w2e[:])
                w2o_t = consts.tile([CHT, K], bf)
                nc.sync.dma_start(w2o_t[:], w2o[:])
                w3e_t = consts.tile([CHT, K], bf)
                nc.sync.dma_start(w3e_t[:], w3e[:])
                w3o_t = consts.tile([CHT, K], bf)
                nc.sync.dma_start(w3o_t[:], w3o[:])
                identB = consts.tile([B, B], bf)
                make_identity(nc, identB)
                zcol = consts.tile([128, 1], dt)
                nc.vector.memset(zcol[:], 0.0)

                conv2_full(0, gos0, new_tab(2, 0))

                # ---- software-pipelined sample loop ----
                for s in range(BL):
                    nxt = s + 1 < BL
                    # table slot rotation in conv1-then-conv2 order (the
                    # 3-slot cycle then always lands writers on slots whose
                    # readers finished an iteration ago)
                    t1n = new_tab(1, s + 2) if s + 2 < BL else None
                    t2n = new_tab(2, s + 1) if nxt else None
                    if nxt:
                        gos2 = [gather_seg(s + 1, "tab1", i)
                                for i in range(3)]
                    if s >= 1:
                        cc_sample(s - 1)
                    if s < BL - 1:
                        gos3 = [gather_seg(s, "tab2", i) for i in range(3)]
                        if s + 2 < BL:
                            load_idx(s + 2)
                            conv1_full(s + 2, t1n)
                        if nxt:
                            conv2_full(s + 1, gos2, t2n)
                        conv3_full(s, gos3)
                        if s == 3:
                            fc1_half(0)
                        elif s == 5:
                            fc1_half(1)
                        elif s == 6:
                            # AllReduce of halves 0,1 (cols 0:32): fired
                            # inside the loop, off the tail critical path
                            y1l1 = work.tile([H1, 32], dt, tag="y1l1")
                            nc.vector.tensor_copy(y1l1[:], y1ps[:, 0:32])
                            nc.sync.dma_start(y1snd1[:], y1l1[:])
                            allreduce(y1snd1, y1rcv1)
                    else:
                        # ---- last sample: half2 fills iter-7's PE slack
                        # (emitted before the gathers so its waits don't get
                        # sem-merged with them); the AllToAll is split in 3
                        # face ranges so the exchange and fc1 half3 overlap
                        # the final gathers.
                        fc1_half(2)
                        g3a = gather_seg(s, "tab2", 0)
                        g3b = gather_seg(s, "tab2", 1)
                        hs3, sink = conv3_sink(s)
                        conv_segs([g3a], w3e_t, w3o_t, sink, [0])
                        nc.sync.dma_start(bounce7[0][:],
                                          hs3[:, S7R[0][0]:S7R[0][1]])
                        g3c = gather_seg(s, "tab2", 2)
                        gos3 = [g3a, g3b, g3c]
                        # triggers sit after all gathers in the Pool queue:
                        # the sequencer runs ahead of the gather engine, so
                        # each fires as soon as its bounce DMA lands.
                        a2a(bounce7[0], recv7[0])
                        fc1_half3(0)
                        conv_segs(gos3, w3e_t, w3o_t, sink, [1])
                        nc.sync.dma_start(bounce7[1][:],
                                          hs3[:, S7R[1][0]:S7R[1][1]])
                        a2a(bounce7[1], recv7[1])
                        fc1_half3(1)
                        conv_segs(gos3, w3e_t, w3o_t, sink, [2])
                        conv_tail(gos3, w3e_t, w3o_t, sink)
                        nc.sync.dma_start(bounce7[2][:],
                                          hs3[:, S7R[2][0]:S7R[2][1]])
                        a2a(bounce7[2], recv7[2])
                        fc1_half3(2)
                        y1l2 = work.tile([H1, 32], dt, tag="y1l2")
                        nc.vector.tensor_copy(y1l2[:], y1ps[:, 32:64])
                        nc.sync.dma_start(y1snd2[:], y1l2[:])
                        allreduce(y1snd2, y1rcv2)
                    st.pop(s)

                # ---- head (replicated) ----
                def bn_relu(y, h, g_ap, b_ap, relu=True):
                    """In-place batchnorm(+relu) on SBUF tile y [h, B]."""
                    mean = work.tile([h, 1], dt, tag=f"bn_m{h}")
                    nc.vector.reduce_sum(mean[:], y[:],
                                         axis=mybir.AxisListType.X)
                    nc.vector.tensor_scalar_mul(mean[:], mean[:], 1.0 / B)
                    sq = work.tile([h, B], dt, tag=f"bn_sq{h}")
                    nc.vector.tensor_tensor(out=sq[:], in0=y[:], in1=y[:],
                                            op=mybir.AluOpType.mult)
                    var = work.tile([h, 1], dt, tag=f"bn_v{h}")
                    nc.vector.reduce_sum(var[:], sq[:],
                                         axis=mybir.AxisListType.X)
                    nc.vector.tensor_scalar_mul(var[:], var[:], 1.0 / B)
                    m2 = work.tile([h, 1], dt, tag=f"bn_m2{h}")
                    nc.vector.tensor_tensor(out=m2[:], in0=mean[:],
                                            in1=mean[:],
                                            op=mybir.AluOpType.mult)
                    nc.vector.tensor_tensor(out=var[:], in0=var[:], in1=m2[:],
                                            op=mybir.AluOpType.subtract)
                    nc.vector.tensor_scalar_add(var[:], var[:], cfg.EPS)
                    std = work.tile([h, 1], dt, tag=f"bn_s{h}")
                    nc.scalar.activation(std[:], var[:],
                                         mybir.ActivationFunctionType.Sqrt,
                                         bias=zcol[:h, :1])
                    rstd = work.tile([h, 1], dt, tag=f"bn_r{h}")
                    nc.vector.reciprocal(rstd[:], std[:])
                    gl = work.tile([h, 1], dt, tag=f"bn_g{h}")
                    nc.sync.dma_start(gl[:], g_ap[:])
                    bl = work.tile([h, 1], dt, tag=f"bn_b{h}")
                    nc.sync.dma_start(bl[:], b_ap[:])
                    scale = work.tile([h, 1], dt, tag=f"bn_sc{h}")
                    nc.vector.tensor_tensor(out=scale[:], in0=rstd[:],
                                            in1=gl[:],
                                            op=mybir.AluOpType.mult)
                    shift = work.tile([h, 1], dt, tag=f"bn_sh{h}")
                    nc.vector.tensor_tensor(out=shift[:], in0=mean[:],
                                            in1=scale[:],
                                            op=mybir.AluOpType.mult)
                    nc.vector.tensor_tensor(out=shift[:], in0=bl[:],
                                            in1=shift[:],
                                            op=mybir.AluOpType.subtract)
                    nc.vector.tensor_scalar(
                        out=y[:], in0=y[:], scalar1=scale[:], scalar2=shift[:],
                        op0=mybir.AluOpType.mult, op1=mybir.AluOpType.add)
                    if relu:
                        nc.scalar.activation(y[:], y[:],
                                             mybir.ActivationFunctionType.Relu,
                                             bias=zcol[:h, :1])

                y1 = work.tile([H1, B], dt, tag="y1h")
                nc.sync.dma_start(y1[:, 0:32], y1rcv1[:])
                nc.sync.dma_start(y1[:, 32:64], y1rcv2[:])
                f1b = work.tile([H1, 1], dt, tag="f1b")
                nc.sync.dma_start(f1b[:], fc1b[:])
                nc.vector.tensor_scalar_add(y1[:], y1[:], f1b[:])
                bn_relu(y1, H1, bn1g, bn1b)

                w2f = work.tile([H1, H2], dt, tag="w2f")
                nc.sync.dma_start(w2f[:], fc2wt[:])
                ps2 = cpsum.tile([K, CHUNK], dt, tag="cp")
                nc.tensor.matmul(out=ps2[0:H2, 0:B], lhsT=w2f[:], rhs=y1[:],
                                 start=True, stop=True)
                y2 = work.tile([H2, B], dt, tag="y2h")
                nc.vector.tensor_copy(y2[:], ps2[0:H2, 0:B])
                f2b = work.tile([H2, 1], dt, tag="f2b")
                nc.sync.dma_start(f2b[:], fc2b[:])
                nc.vector.tensor_scalar_add(y2[:], y2[:], f2b[:])
                bn_relu(y2, H2, bn2g, bn2b)

                wof = work.tile([H2, NCLS], dt, tag="wof")
                nc.sync.dma_start(wof[:], fcowt[:])
                pso = cpsum.tile([K, CHUNK], dt, tag="cp")
                nc.tensor.matmul(out=pso[0:NCLS, 0:B], lhsT=wof[:], rhs=y2[:],
                                 start=True, stop=True)
                yo = work.tile([NCLS, B], dt, tag="yo")
                nc.vector.tensor_copy(yo[:], pso[0:NCLS, 0:B])
                fob = work.tile([NCLS, 1], dt, tag="fob")
                nc.sync.dma_start(fob[:], fcob[:])
                nc.vector.tensor_scalar_add(yo[:], yo[:], fob[:])
                nc.sync.dma_start(out[:], yo[:])

    nc.compile()
    return nc


_CACHE: dict = {}


def _get_program(cfg: Cfg):
    key = cfg
    if key not in _CACHE:
        _CACHE[key] = build_program(cfg)
    return _CACHE[key]


def kernel(**inputs) -> np.ndarray:
    from concourse import bass_utils

    cfg = CFG
    nc = _get_program(cfg)
    in_maps = prep_core_inputs(cfg, **inputs)
    res = bass_utils.run_bass_kernel_spmd(
        nc, in_maps, core_ids=list(range(cfg.ncores)))
    return postprocess(res.results[0]["out"], cfg)


# revision 26
# speedup vs baseline: 1.2167x; 1.0035x over previous
"""Trainium2 Bass kernel for nn_CNN_9818295238933 (gnn_message_passing).

Data-parallel over batch across 8 cores (8 samples each). Per sample:
  conv1 (PE, bf16) -> h1 [32, F] -> REP matmul replicates h1 across 8
  partition groups as a bf16-pair-packed SBUF table [128, F] (partition
  (g, kp) holds the bf16 pair (h[2kp], h[2kp+1]) at face f).
  ap_gather (GPSIMD, SBUF-local) gathers the table with that sample's
  adjacency: groups 0-6 carry neighbour slot n for faces [0, FL); group 7
  carries the tail faces [FL, FG) of all 7 slots concatenated, so all 8
  Q7 cores work and each instruction processes FL/seg indices instead of
  FG. Gathered tiles feed the next conv directly as strided bf16 matmul
  rhs (contraction over (n, kp) partitions, even/odd k accumulated in
  PSUM); tail faces get per-n 16-partition matmuls from group 7's slice.
  Repeat for conv2 -> table2 -> gather -> conv3.

The ap_gather ucode is the hard bottleneck (~27.3 ns/index per Q7 core,
measured on idle HW; 48 segment-gathers x ~71.3 us = 3.42 ms). The
sample loop is software-pipelined so the Pool engine never waits:
gathers are emitted as [g2(s+1) segs][cc(s-1)][g3(s) segs]; convs are
emitted consumer-first (conv2(s+1), conv3(s) BEFORE conv1(s+2)) so PE
frees gather buffers promptly. All three tables (tab1 x2 live +
tab2 x1) share one tag-rotated 3-slot pool, which double-buffers tab1
at startup (kills the prologue stall) while keeping baseline SBUF use.

Head: idx(0)+conv1(0) emitted before all other constants. Tail: h3
bounces to DRAM per sample and a per-sample AllToAll redistributes
k-slices; fc1 runs as 3 16-col halves (iters 3/5/6) + sample-6 and
sample-7 8-col pieces; sample 7's AllToAll is split at face 2624 so
its first third (and fc1 piece7A) overlaps the final gather segment;
AllReduce is split in 3 column groups (2 fire during the loop).
BN+ReLU+fc2+BN+ReLU+fco replicated.

Self-contained: hardcodes all shapes; only imports the Trainium toolchain.
"""

import sys
from dataclasses import dataclass

if "/opt/trn_rl_repo" not in sys.path:
    sys.path.insert(0, "/opt/trn_rl_repo")

import numpy as np


@dataclass(frozen=True)
class Cfg:
    ncores: int = 8
    B: int = 64
    C: int = 12
    N: int = 7
    K: int = 32
    F: int = 9000
    FG: int = 9008          # compute/table extent (F padded to mult of 16)
    FL: int = 7888          # main faces per neighbour group (= FG * 7/8 pad16)
    H1: int = 100
    H2: int = 30
    NCLS: int = 2
    EPS: float = 1e-5
    CHUNK: int = 512        # PSUM f-chunk

    @property
    def BL(self):
        return self.B // self.ncores

    @property
    def CN(self):
        return self.C * self.N

    @property
    def KL(self):
        return self.K // self.ncores

    @property
    def KP(self):
        return self.K // 2

    @property
    def CHT(self):
        return self.N * self.KP  # 112 main channels

    @property
    def TL(self):
        return self.FG - self.FL  # 1120 tail faces

    @property
    def SEGS(self):
        # Segment starts must be multiples of 32 entries: the gather
        # ucode reads the wrapped idx list as u32 words, and a 2-byte
        # misaligned base corrupts words crossing 16-byte boundaries.
        if self.FL == 9008:  # tail disabled
            return [(0, 3008), (3008, 3008), (6016, 2992)]
        return [(0, 2624), (2624, 2624), (5248, 2640)]

    @property
    def WCOL(self):
        return self.FL // 16  # wrapped idx columns (493)

    @property
    def SPLIT(self):
        return self.SEGS[1][0]  # face split for sample-7's early AllToAll


CFG = Cfg()


def _chunks(f0, flen, step):
    out = []
    f = f0
    while f < f0 + flen:
        out.append((f, min(step, f0 + flen - f)))
        f += step
    return out


# ---------------------------------------------------------------------------
# Host-side input preparation
# ---------------------------------------------------------------------------

def prep_core_inputs(cfg: Cfg, x, adjacencies, W1, W2, W3, fc1_w, fc1_b, bn1_g,
                     bn1_b, fc2_w, fc2_b, bn2_g, bn2_b, fco_w, fco_b):
    import ml_dtypes
    bf16 = ml_dtypes.bfloat16

    B, C, N, K, F, FG, FL = (cfg.B, cfg.C, cfg.N, cfg.K, cfg.F, cfg.FG,
                             cfg.FL)
    BL, CN, KL, KP, TL = cfg.BL, cfg.CN, cfg.KL, cfg.KP, cfg.TL
    H1, H2, NCLS = cfg.H1, cfg.H2, cfg.NCLS

    x = np.asarray(x, dtype=np.float32)
    adj = np.asarray(adjacencies).astype(np.int64)[:, 0]  # [B, F, N]

    # x [B, C, F, N] -> xt [B, (c,n), FG] bf16, zero-padded along f.
    xt = np.zeros((B, CN, FG), dtype=bf16)
    xt[:, :, :F] = np.transpose(x, (0, 1, 3, 2)).reshape(B, CN, F).astype(bf16)

    # Gather index lists, one per 16-partition group:
    #   group n < 7: adj[b, f, n] for f in [0, FL)
    #   group 7:     adj[b, FL+u, n] at position n*TL+u (pad to FL with 0)
    # wrapped so entry i sits at [16g + i%16, i//16]. Segment boundaries
    # are multiples of 16 so column-slicing yields each segment's list.
    idx_pad = np.zeros((B, FG, N), dtype=np.int64)
    idx_pad[:, :F] = adj
    lists = np.zeros((B, 8, FL), dtype=np.int64)
    lists[:, :7, :] = np.transpose(idx_pad[:, :FL], (0, 2, 1))
    lists[:, 7, :N * TL] = np.transpose(
        idx_pad[:, FL:], (0, 2, 1)).reshape(B, N * TL)
    wrap = lists.reshape(B, 8, FL // 16, 16)
    idx16 = np.ascontiguousarray(
        np.transpose(wrap, (0, 1, 3, 2)).reshape(B, 128, FL // 16)
    ).astype(np.int16)

    w1f = np.transpose(np.asarray(W1, np.float32), (1, 2, 0)).reshape(CN, K)

    def eo(Wm):  # [K_out, K_in, N] -> even/odd lhsT [(n,kp), K_out] bf16
        Wm = np.asarray(Wm, np.float32)
        we = np.transpose(Wm[:, 0::2, :], (2, 1, 0)).reshape(N * KP, K)
        wo = np.transpose(Wm[:, 1::2, :], (2, 1, 0)).reshape(N * KP, K)
        return (np.ascontiguousarray(we).astype(bf16),
                np.ascontiguousarray(wo).astype(bf16))

    w2e, w2o = eo(W2)
    w3e, w3o = eo(W3)

    # Replication matrices over all 8 groups: repe[q, (g,kp)] = (q == 2*kp)
    q = np.arange(K)[:, None]
    p = np.arange(128)[None, :]
    repe = (q == 2 * (p % KP)).astype(bf16)
    repo = (q == 2 * (p % KP) + 1).astype(bf16)

    # fc1 weights: [H1, K*F] -> [K, FG, H1] zero-padded, per-core k-slice.
    fc1 = np.asarray(fc1_w, np.float32).reshape(H1, K, F)
    fc1t = np.zeros((K, FG, H1), dtype=bf16)
    fc1t[:, :F] = np.transpose(fc1, (1, 2, 0)).astype(bf16)

    fc2wt = np.ascontiguousarray(np.asarray(fc2_w, np.float32).T)  # [H1, H2]
    fcowt = np.ascontiguousarray(np.asarray(fco_w, np.float32).T)  # [H2, NCLS]

    def col(v, n):
        return np.asarray(v, np.float32).reshape(n, 1)

    shared = dict(
        w1=w1f.astype(bf16), w2e=w2e, w2o=w2o, w3e=w3e, w3o=w3o,
        repe=repe, repo=repo,
        fc1b=col(fc1_b, H1), bn1g=col(bn1_g, H1), bn1b=col(bn1_b, H1),
        fc2wt=fc2wt, fc2b=col(fc2_b, H2), bn2g=col(bn2_g, H2),
        bn2b=col(bn2_b, H2), fcowt=fcowt, fcob=col(fco_b, NCLS),
    )

    in_maps = []
    for c in range(cfg.ncores):
        bsl = slice(c * BL, (c + 1) * BL)
        fc1wt_c = np.ascontiguousarray(
            fc1t[c * KL:(c + 1) * KL].reshape(KL * FG, H1))
        m = dict(shared)
        m.update(
            xt=np.ascontiguousarray(xt[bsl]),
            idx16=np.ascontiguousarray(idx16[bsl]),
            fc1wt=fc1wt_c,
        )
        in_maps.append(m)
    return in_maps


def postprocess(out_dev: np.ndarray, cfg: Cfg = CFG) -> np.ndarray:
    """Device out columns are (sample-within-core, core) ordered; return
    [B, NCLS] in global sample order (core-major)."""
    o = np.asarray(out_dev, np.float32).reshape(cfg.NCLS, cfg.BL, cfg.ncores)
    return np.ascontiguousarray(o.transpose(2, 1, 0).reshape(cfg.B, cfg.NCLS))


# ---------------------------------------------------------------------------
# Device program
# ---------------------------------------------------------------------------

def build_program(cfg: Cfg):
    import concourse.bass as bass  # noqa: F401
    import concourse.bacc as bacc
    import concourse.mybir as mybir
    import concourse.tile as tile
    from concourse.masks import make_identity

    dt = mybir.dt.float32
    bf = mybir.dt.bfloat16
    u32 = mybir.dt.uint32
    i16 = mybir.dt.int16
    B, C, N, K, FG, FL = cfg.B, cfg.C, cfg.N, cfg.K, cfg.FG, cfg.FL
    BL, CN, KL, KP, CHT, TL = (cfg.BL, cfg.CN, cfg.KL, cfg.KP, cfg.CHT,
                               cfg.TL)
    H1, H2, NCLS = cfg.H1, cfg.H2, cfg.NCLS
    CHUNK, SEGS, WCOL = cfg.CHUNK, cfg.SEGS, cfg.WCOL
    SPLIT = cfg.SPLIT
    NCORES = cfg.ncores
    SEGMAX = max(w for _, w in SEGS)
    BLK = 1024
    rg = [list(range(NCORES))]

    nc = bacc.Bacc("TRN2", target_bir_lowering=False, debug=False,
                   num_devices=NCORES, num_swdge_queues=4)

    xt = nc.dram_tensor("xt", [BL, CN, FG], bf, kind="ExternalInput")
    idx16 = nc.dram_tensor("idx16", [BL, 128, WCOL], i16,
                           kind="ExternalInput")
    w1 = nc.dram_tensor("w1", [CN, K], bf, kind="ExternalInput")
    w2e = nc.dram_tensor("w2e", [CHT, K], bf, kind="ExternalInput")
    w2o = nc.dram_tensor("w2o", [CHT, K], bf, kind="ExternalInput")
    w3e = nc.dram_tensor("w3e", [CHT, K], bf, kind="ExternalInput")
    w3o = nc.dram_tensor("w3o", [CHT, K], bf, kind="ExternalInput")
    repe = nc.dram_tensor("repe", [K, 128], bf, kind="ExternalInput")
    repo = nc.dram_tensor("repo", [K, 128], bf, kind="ExternalInput")
    fc1wt = nc.dram_tensor("fc1wt", [KL * FG, H1], bf, kind="ExternalInput")
    fc1b = nc.dram_tensor("fc1b", [H1, 1], dt, kind="ExternalInput")
    bn1g = nc.dram_tensor("bn1g", [H1, 1], dt, kind="ExternalInput")
    bn1b = nc.dram_tensor("bn1b", [H1, 1], dt, kind="ExternalInput")
    fc2wt = nc.dram_tensor("fc2wt", [H1, H2], dt, kind="ExternalInput")
    fc2b = nc.dram_tensor("fc2b", [H2, 1], dt, kind="ExternalInput")
    bn2g = nc.dram_tensor("bn2g", [H2, 1], dt, kind="ExternalInput")
    bn2b = nc.dram_tensor("bn2b", [H2, 1], dt, kind="ExternalInput")
    fcowt = nc.dram_tensor("fcowt", [H2, NCLS], dt, kind="ExternalInput")
    fcob = nc.dram_tensor("fcob", [NCLS, 1], dt, kind="ExternalInput")
    out = nc.dram_tensor("out", [NCLS, B], dt, kind="ExternalOutput")

    def tail_pieces():
        """(n, seg_idx, seg_local_start, width, tail_local_start) pieces
        covering each neighbour's [n*TL, (n+1)*TL) slice of group-7's
        entry list, split at gather-segment boundaries."""
        out_runs = []
        for n in range(N):
            e0, e1 = n * TL, (n + 1) * TL
            for si, (s0, slen) in enumerate(SEGS):
                lo = max(e0, s0)
                hi = min(e1, s0 + slen)
                if lo < hi:
                    out_runs.append((n, si, lo - s0, hi - lo, lo - e0))
        return out_runs

    with tile.TileContext(nc) as tc:
        with (
            tc.tile_pool(name="consts", bufs=1) as consts,
            tc.tile_pool(name="xcp", bufs=2) as xcp,
            tc.tile_pool(name="idxp", bufs=3) as idxp,
            tc.tile_pool(name="tabs", bufs=3) as tabsp,
            tc.tile_pool(name="gop", bufs=4) as gop,
            tc.tile_pool(name="tbp", bufs=1) as tbp,
            tc.tile_pool(name="hp", bufs=1) as hp,
            tc.tile_pool(name="hst", bufs=2) as hstp,
            tc.tile_pool(name="work", bufs=2) as work,
            tc.tile_pool(name="xbp", bufs=2) as xbp,
            tc.tile_pool(name="dram", bufs=1, space="DRAM") as dram,
        ):
            # ---- constants needed by conv1(0) go first; the rest are
            # emitted after the first gathers so they don't delay the head.
            w1_t = consts.tile([CN, K], bf)
            nc.sync.dma_start(w1_t[:], w1[:])
            repe_t = consts.tile([K, 128], bf)
            nc.sync.dma_start(repe_t[:], repe[:])
            repo_t = consts.tile([K, 128], bf)
            nc.sync.dma_start(repo_t[:], repo[:])

            bounce = dram.tile([BL, NCORES, KL, FG], bf)
            recv = dram.tile([BL, NCORES, KL, FG], bf)
            # sample-7 AllToAll face-range splits (per gather segment, then
            # the tail faces) so each fires as soon as its conv3 part lands
            S7R = [(0, SEGS[1][0]), (SEGS[1][0], SEGS[2][0]),
                   (SEGS[2][0], FL), (FL, FG)]
            bounce7 = [dram.tile([NCORES, KL, hi - lo], bf,
                                 name=f"bounce7_{i}")
                       for i, (lo, hi) in enumerate(S7R)]
            recv7 = [dram.tile([NCORES, KL, hi - lo], bf,
                               name=f"recv7_{i}")
                     for i, (lo, hi) in enumerate(S7R)]
            y1snd1 = dram.tile([H1, 32], dt)
            y1rcv1 = dram.tile([H1, 32], dt)
            y1snd2 = dram.tile([H1, 32], dt)
            y1rcv2 = dram.tile([H1, 32], dt)

            with (
                tc.tile_pool(name="cpsum", bufs=2, space="PSUM") as cpsum,
                tc.tile_pool(name="rpsum", bufs=2, space="PSUM") as rpsum,
                tc.tile_pool(name="fpsum", bufs=1, space="PSUM") as fpsum,
            ):
                def build_table(tab, hs, f0, w):
                    """REP-matmul an h chunk [K, w] into the packed table."""
                    tb = tab[:].bitcast(bf).rearrange(
                        "p (f two) -> p f two", two=2)
                    pse = rpsum.tile([128, CHUNK], dt, tag="rp")
                    nc.tensor.matmul(out=pse[:, :w], lhsT=repe_t[:],
                                     rhs=hs[:, :w], start=True, stop=True)
                    nc.vector.tensor_copy(tb[:, f0:f0 + w, 0], pse[:, :w])
                    pso = rpsum.tile([128, CHUNK], dt, tag="rp")
                    nc.tensor.matmul(out=pso[:, :w], lhsT=repo_t[:],
                                     rhs=hs[:, :w], start=True, stop=True)
                    nc.scalar.copy(tb[:, f0:f0 + w, 1], pso[:, :w])

                st = {}  # per-sample tile state

                def load_idx(s):
                    idx_t = idxp.tile([128, WCOL], i16, tag="it")
                    nc.sync.dma_start(idx_t[:], idx16[s])
                    st[s] = dict(idx=idx_t)

                XBLK = 2252  # x staged in 4 big DMAs instead of 18 small

                def conv1_full(s, tab1):
                    st[s]["tab1"] = tab1
                    for xb0, xbw in _chunks(0, FG, XBLK):
                        xc = xbp.tile([CN, XBLK], bf, tag="xb")
                        nc.sync.dma_start(xc[:, :xbw],
                                          xt[s, :, xb0:xb0 + xbw])
                        for f0, w in _chunks(xb0, xbw, CHUNK):
                            lo = f0 - xb0
                            ps = cpsum.tile([K, CHUNK], dt, tag="cp")
                            nc.tensor.matmul(out=ps[:, :w], lhsT=w1_t[:],
                                             rhs=xc[:, lo:lo + w],
                                             start=True, stop=True)
                            hs = hstp.tile([K, CHUNK], bf, tag="hst")
                            nc.vector.tensor_copy(hs[:, :w], ps[:, :w])
                            build_table(tab1, hs, f0, w)

                def gather_seg(s, tab_key, seg):
                    s0, slen = SEGS[seg]
                    go = gop.tile([128, SEGMAX], u32, tag="go")
                    nc.gpsimd.ap_gather(
                        out_ap=go[:, :slen], in_ap=st[s][tab_key][:],
                        idxs_ap=st[s]["idx"][:, s0 // 16:(s0 + slen) // 16],
                        channels=128, num_elems=FG, d=1, num_idxs=slen)
                    return go

                def stage_tail(gos):
                    """SBUF->SBUF DMA group-7's gathered entries into the
                    main (n, kp) partition layout: tailbuf[16n+kp, u] =
                    go[112+kp, n*TL+u]. Returns the [CHT, TL] u32 tile."""
                    tb = tbp.tile([CHT, TL], u32, tag="tb")
                    for n, si, lo, rw, u0 in tail_pieces():
                        nc.sync.dma_start(
                            tb[16 * n:16 * n + 16, u0:u0 + rw],
                            gos[si][112:128, lo:lo + rw])
                    return tb

                def conv_segs(gos, we_t, wo_t, sink, seg_ids):
                    """Conv faces covered by the given gather segments."""
                    for si in seg_ids:
                        s0, slen = SEGS[si]
                        gb = gos[si][:, :slen].bitcast(bf).rearrange(
                            "p (f two) -> p f two", two=2)
                        for f0, w in _chunks(s0, slen, CHUNK):
                            lo = f0 - s0
                            ps = cpsum.tile([K, CHUNK], dt, tag="cp")
                            nc.tensor.matmul(out=ps[:, :w],
                                             lhsT=we_t[:],
                                             rhs=gb[0:CHT, lo:lo + w, 0],
                                             start=True, stop=False)
                            nc.tensor.matmul(out=ps[:, :w],
                                             lhsT=wo_t[:],
                                             rhs=gb[0:CHT, lo:lo + w, 1],
                                             start=False, stop=True)
                            sink(f0, w, ps)

                def conv_tail(gos, we_t, wo_t, sink):
                    """Conv the tail faces [FL, FG) via the restaged
                    group-7 entries."""
                    if FL >= FG:
                        return
                    tb = stage_tail(gos)
                    tbb = tb[:].bitcast(bf).rearrange(
                        "p (f two) -> p f two", two=2)
                    for f0, w in _chunks(FL, FG - FL, CHUNK):
                        lo = f0 - FL
                        ps = cpsum.tile([K, CHUNK], dt, tag="cp")
                        nc.tensor.matmul(out=ps[:, :w], lhsT=we_t[:],
                                         rhs=tbb[:, lo:lo + w, 0],
                                         start=True, stop=False)
                        nc.tensor.matmul(out=ps[:, :w], lhsT=wo_t[:],
                                         rhs=tbb[:, lo:lo + w, 1],
                                         start=False, stop=True)
                        sink(f0, w, ps)

                def conv2_full(s, gos, tab2):
                    st[s]["tab2"] = tab2

                    def sink(f0, w, ps):
                        hs = hstp.tile([K, CHUNK], bf, tag="hst")
                        nc.vector.tensor_copy(hs[:, :w], ps[:, :w])
                        build_table(tab2, hs, f0, w)
                    conv_segs(gos, w2e_t, w2o_t, sink, [0, 1, 2])
                    conv_tail(gos, w2e_t, w2o_t, sink)

                def conv3_sink(s):
                    hs3 = hp.tile([K, FG], bf, tag="h3", name=f"h3_{s}")

                    def sink(f0, w, ps):
                        nc.vector.tensor_copy(hs3[:, f0:f0 + w], ps[:, :w])
                    return hs3, sink

                def conv3_full(s, gos):
                    hs3, sink = conv3_sink(s)
                    conv_segs(gos, w3e_t, w3o_t, sink, [0, 1, 2])
                    conv_tail(gos, w3e_t, w3o_t, sink)
                    # single bounce write per sample: the AllToAll's input
                    # must have one writer (chunked writers race the
                    # collective on HW).
                    nc.sync.dma_start(bounce[s], hs3[:])

                def cc_sample(s):
                    nc.gpsimd.collective_compute(
                        "AllToAll", mybir.AluOpType.bypass,
                        replica_groups=rg,
                        ins=[bounce[s].opt()], outs=[recv[s].opt()])

                # ---- fc1: y1ps[:, cols] += fc1wt.T @ transposed recv rows,
                # accumulated per column group over KL x 71 PSUM chunks.
                y1ps = fpsum.tile([H1, B], dt, tag="y1")
                fc1_state = {}

                def fc1_part(grp, c0, ncols, bblocks, load_rows, total_nst):
                    stt = fc1_state.setdefault(grp, dict(stp=0))
                    for kl in range(KL):
                        for b0, bw in bblocks:
                            lt_in = work.tile([ncols, BLK], bf,
                                              tag=f"ltin{ncols}")
                            load_rows(lt_in, kl, b0, bw)
                            r0 = kl * FG + b0
                            nfull = bw // 128
                            wt = work.tile([128, (BLK // 128) * H1], bf,
                                           tag="fw")
                            if nfull:
                                nc.scalar.dma_start(
                                    wt[:, :nfull * H1].rearrange(
                                        "p (c h) -> p c h", h=H1),
                                    fc1wt[r0:r0 + nfull * 128, :].rearrange(
                                        "(c p) h -> p c h", p=128))
                            for ci, (s0c, wc) in enumerate(
                                    _chunks(0, bw, 128)):
                                pst = rpsum.tile([128, 16], bf, tag="tT")
                                nc.tensor.transpose(
                                    pst[:wc, :ncols],
                                    lt_in[:, s0c:s0c + wc],
                                    identB[:ncols, :ncols])
                                ltt = work.tile([128, ncols], bf,
                                                tag=f"ltt{ncols}")
                                nc.vector.tensor_copy(ltt[:wc, :],
                                                      pst[:wc, :ncols])
                                if wc == 128:
                                    lhsT = wt[:, ci * H1:(ci + 1) * H1]
                                else:
                                    wtp = work.tile([128, H1], bf, tag="fwp")
                                    nc.scalar.dma_start(
                                        wtp[:wc, :],
                                        fc1wt[r0 + s0c:r0 + s0c + wc, :])
                                    lhsT = wtp[:wc, :]
                                nc.tensor.matmul(
                                    out=y1ps[:, c0:c0 + ncols],
                                    lhsT=lhsT, rhs=ltt[:wc, :],
                                    start=(stt["stp"] == 0),
                                    stop=(stt["stp"] == total_nst - 1))
                                stt["stp"] += 1

                FULL_BLOCKS = _chunks(0, FG, BLK)
                NST_FULL = KL * sum(len(_chunks(0, bw, 128))
                                    for _, bw in FULL_BLOCKS)

                def fc1_half(h):
                    def load_rows(t, kl, b0, bw):
                        nc.scalar.dma_start(
                            t[:, :bw], recv[2 * h:2 * h + 2, :, kl,
                                            b0:b0 + bw])
                    fc1_part(f"h{h}", 16 * h, 16, FULL_BLOCKS, load_rows,
                             NST_FULL)

                BLOCKS_7 = [_chunks(lo, hi - lo, BLK) for lo, hi in S7R]
                NST_P7 = KL * sum(len(_chunks(0, bw, 128))
                                  for blocks in BLOCKS_7
                                  for _, bw in blocks)

                def fc1_half3(part):
                    """cols 48:64 = samples 6,7; face-range `part` so each
                    part unblocks as soon as its sample-7 AllToAll lands."""
                    src, off = recv7[part], S7R[part][0]

                    def load_rows(t, kl, b0, bw):
                        nc.scalar.dma_start(
                            t[0:8, :bw], recv[6:7, :, kl, b0:b0 + bw])
                        nc.scalar.dma_start(
                            t[8:16, :bw],
                            src[:, kl, b0 - off:b0 - off + bw])
                    fc1_part("h3", 48, 16, BLOCKS_7[part], load_rows, NST_P7)

                def new_tab(kind, s):
                    return tabsp.tile([128, FG], u32, tag="tab",
                                      name=f"tab{kind}_{s}")

                def a2a(src, dst):
                    nc.gpsimd.collective_compute(
                        "AllToAll", mybir.AluOpType.bypass,
                        replica_groups=rg,
                        ins=[src[:].opt()], outs=[dst[:].opt()])

                def allreduce(src, dst):
                    nc.gpsimd.collective_compute(
                        "AllReduce", mybir.AluOpType.add, replica_groups=rg,
                        ins=[src[:].opt()], outs=[dst[:].opt()])

                # ---- prologue: sample-0 table + first gathers before all
                # other constants, so the head is just idx0+conv1(0).
                load_idx(0)
                # keep the (otherwise unused) xcp pool's footprint so the
                # SBUF layout of the pools behind it doesn't shift — the
                # ap_gather ucode rate is sensitive to table placement.
                xpad0 = xcp.tile([CN, CHUNK], bf, tag="xc")
                nc.vector.memset(xpad0[:, :8], 0.0)
                xpad1 = xcp.tile([CN, CHUNK], bf, tag="xc")
                nc.vector.memset(xpad1[:, :8], 0.0)
                # 32-index dummy gather (full 128 channels, like the real
                # ones — a 16-channel dummy flips the ucode into a ~20%
                # slower mode for the whole run): starts the ~70us Q7
                # library IRAM load now, overlapped with conv1(0).
                didx = idxp.tile([128, WCOL], i16, tag="it")
                nc.vector.memset(didx[:, 0:8], 0.0)
                dout = gop.tile([128, SEGMAX], u32, tag="go")
                nc.gpsimd.ap_gather(
                    out_ap=dout[:, :32],
                    in_ap=dout[:, 2000:2240],
                    idxs_ap=didx[:, 0:2],
                    channels=128, num_elems=240, d=1, num_idxs=32)
                conv1_full(0, new_tab(1, 0))
                gos0 = [gather_seg(0, "tab1", i) for i in range(3)]
                load_idx(1)
                conv1_full(1, new_tab(1, 1))

                w2e_t = consts.tile([CHT, K], bf)
                nc.sync.dma_start(w2e_t[:], w2e[:])
                w2o_t = consts.tile([CHT, K], bf)
                nc.sync.dma_start(w2o_t[:], w2o[:])
                w3e_t = consts.tile([CHT, K], bf)
                nc.sync.dma_start(w3e_t[:], w3e[:])
                w3o_t = consts.tile([CHT, K], bf)
                nc.sync.dma_start(w3o_t[:], w3o[:])
                identB = consts.tile([B, B], bf)
                make_identity(nc, identB)
                zcol = consts.tile([128, 1], dt)
                nc.vector.memset(zcol[:], 0.0)

                conv2_full(0, gos0, new_tab(2, 0))

                # ---- software-pipelined sample loop ----
                for s in range(BL):
                    nxt = s + 1 < BL
                    # table slot rotation in conv1-then-conv2 order (the
                    # 3-slot cycle then always lands writers on slots whose
                    # readers finished an iteration ago)
                    t1n = new_tab(1, s + 2) if s + 2 < BL else None
                    t2n = new_tab(2, s + 1) if nxt else None
                    if nxt:
                        gos2 = [gather_seg(s + 1, "tab1", i)
                                for i in range(3)]
                    if s >= 1:
                        cc_sample(s - 1)
                    if s < BL - 1:
                        gos3 = [gather_seg(s, "tab2", i) for i in range(3)]
                        if s + 2 < BL:
                            load_idx(s + 2)
                            conv1_full(s + 2, t1n)
                        if nxt:
                            conv2_full(s + 1, gos2, t2n)
                        conv3_full(s, gos3)
                        if s == 3:
                            fc1_half(0)
                        elif s == 5:
                            fc1_half(1)
                        elif s == 6:
                            # AllReduce of halves 0,1 (cols 0:32): fired
                            # inside the loop, off the tail critical path
                            y1l1 = work.tile([H1, 32], dt, tag="y1l1")
                            nc.vector.tensor_copy(y1l1[:], y1ps[:, 0:32])
                            nc.sync.dma_start(y1snd1[:], y1l1[:])
                            allreduce(y1snd1, y1rcv1)
                    else:
                        # ---- last sample: half2 fills iter-7's PE slack
                        # (emitted before the gathers so its waits don't get
                        # sem-merged with them); the AllToAll is split in 3
                        # face ranges so the exchange and fc1 half3 overlap
                        # the final gathers.
                        fc1_half(2)
                        g3a = gather_seg(s, "tab2", 0)
                        g3b = gather_seg(s, "tab2", 1)
                        hs3, sink = conv3_sink(s)
                        conv_segs([g3a], w3e_t, w3o_t, sink, [0])
                        nc.sync.dma_start(bounce7[0][:],
                                          hs3[:, S7R[0][0]:S7R[0][1]])
                        g3c = gather_seg(s, "tab2", 2)
                        gos3 = [g3a, g3b, g3c]
                        # triggers sit after all gathers in the Pool queue:
                        # the sequencer runs ahead of the gather engine, so
                        # each fires as soon as its bounce DMA lands.
                        a2a(bounce7[0], recv7[0])
                        fc1_half3(0)
                        conv_segs(gos3, w3e_t, w3o_t, sink, [1])
                        nc.sync.dma_start(bounce7[1][:],
                                          hs3[:, S7R[1][0]:S7R[1][1]])
                        a2a(bounce7[1], recv7[1])
                        fc1_half3(1)
                        conv_segs(gos3, w3e_t, w3o_t, sink, [2])
                        nc.sync.dma_start(bounce7[2][:],
                                          hs3[:, S7R[2][0]:S7R[2][1]])
                        a2a(bounce7[2], recv7[2])
                        conv_tail(gos3, w3e_t, w3o_t, sink)
                        nc.sync.dma_start(bounce7[3][:],
                                          hs3[:, S7R[3][0]:S7R[3][1]])
                        a2a(bounce7[3], recv7[3])
                        fc1_half3(2)
                        fc1_half3(3)
                        y1l2 = work.tile([H1, 32], dt, tag="y1l2")
                        nc.vector.tensor_copy(y1l2[:], y1ps[:, 32:64])
                        nc.sync.dma_start(y1snd2[:], y1l2[:])
                        allreduce(y1snd2, y1rcv2)
                    st.pop(s)

                # ---- head (replicated) ----
                def bn_relu(y, h, g_ap, b_ap, relu=True):
                    """In-place batchnorm(+relu) on SBUF tile y [h, B]."""
                    mean = work.tile([h, 1], dt, tag=f"bn_m{h}")
                    nc.vector.reduce_sum(mean[:], y[:],
                                         axis=mybir.AxisListType.X)
                    nc.vector.tensor_scalar_mul(mean[:], mean[:], 1.0 / B)
                    sq = work.tile([h, B], dt, tag=f"bn_sq{h}")
                    nc.vector.tensor_tensor(out=sq[:], in0=y[:], in1=y[:],
                                            op=mybir.AluOpType.mult)
                    var = work.tile([h, 1], dt, tag=f"bn_v{h}")
                    nc.vector.reduce_sum(var[:], sq[:],
                                         axis=mybir.AxisListType.X)
                    nc.vector.tensor_scalar_mul(var[:], var[:], 1.0 / B)
                    m2 = work.tile([h, 1], dt, tag=f"bn_m2{h}")
                    nc.vector.tensor_tensor(out=m2[:], in0=mean[:],
                                            in1=mean[:],
                                            op=mybir.AluOpType.mult)
                    nc.vector.tensor_tensor(out=var[:], in0=var[:], in1=m2[:],
                                            op=mybir.AluOpType.subtract)
                    nc.vector.tensor_scalar_add(var[:], var[:], cfg.EPS)
                    std = work.tile([h, 1], dt, tag=f"bn_s{h}")
                    nc.scalar.activation(std[:], var[:],
                                         mybir.ActivationFunctionType.Sqrt,
                                         bias=zcol[:h, :1])
                    rstd = work.tile([h, 1], dt, tag=f"bn_r{h}")
                    nc.vector.reciprocal(rstd[:], std[:])
                    gl = work.tile([h, 1], dt, tag=f"bn_g{h}")
                    nc.sync.dma_start(gl[:], g_ap[:])
                    bl = work.tile([h, 1], dt, tag=f"bn_b{h}")
                    nc.sync.dma_start(bl[:], b_ap[:])
                    scale = work.tile([h, 1], dt, tag=f"bn_sc{h}")
                    nc.vector.tensor_tensor(out=scale[:], in0=rstd[:],
                                            in1=gl[:],
                                            op=mybir.AluOpType.mult)
                    shift = work.tile([h, 1], dt, tag=f"bn_sh{h}")
                    nc.vector.tensor_tensor(out=shift[:], in0=mean[:],
                                            in1=scale[:],
                                            op=mybir.AluOpType.mult)
                    nc.vector.tensor_tensor(out=shift[:], in0=bl[:],
                                            in1=shift[:],
                                            op=mybir.AluOpType.subtract)
                    nc.vector.tensor_scalar(
                        out=y[:], in0=y[:], scalar1=scale[:], scalar2=shift[:],
                        op0=mybir.AluOpType.mult, op1=mybir.AluOpType.add)
                    if relu:
                        nc.scalar.activation(y[:], y[:],
                                             mybir.ActivationFunctionType.Relu,
                                             bias=zcol[:h, :1])

                y1 = work.tile([H1, B], dt, tag="y1h")
                nc.sync.dma_start(y1[:, 0:32], y1rcv1[:])
                nc.sync.dma_start(y1[:, 32:64], y1rcv2[:])
                f1b = work.tile([H1, 1], dt, tag="f1b")
                nc.sync.dma_start(f1b[:], fc1b[:])
                nc.vector.tensor_scalar_add(y1[:], y1[:], f1b[:])
                bn_relu(y1, H1, bn1g, bn1b)

                w2f = work.tile([H1, H2], dt, tag="w2f")
                nc.sync.dma_start(w2f[:], fc2wt[:])
                ps2 = cpsum.tile([K, CHUNK], dt, tag="cp")
                nc.tensor.matmul(out=ps2[0:H2, 0:B], lhsT=w2f[:], rhs=y1[:],
                                 start=True, stop=True)
                y2 = work.tile([H2, B], dt, tag="y2h")
                nc.vector.tensor_copy(y2[:], ps2[0:H2, 0:B])
                f2b = work.tile([H2, 1], dt, tag="f2b")
                nc.sync.dma_start(f2b[:], fc2b[:])
                nc.vector.tensor_scalar_add(y2[:], y2[:], f2b[:])
                bn_relu(y2, H2, bn2g, bn2b)

                wof = work.tile([H2, NCLS], dt, tag="wof")
                nc.sync.dma_start(wof[:], fcowt[:])
                pso = cpsum.tile([K, CHUNK], dt, tag="cp")
                nc.tensor.matmul(out=pso[0:NCLS, 0:B], lhsT=wof[:], rhs=y2[:],
                                 start=True, stop=True)
                yo = work.tile([NCLS, B], dt, tag="yo")
                nc.vector.tensor_copy(yo[:], pso[0:NCLS, 0:B])
                fob = work.tile([NCLS, 1], dt, tag="fob")
                nc.sync.dma_start(fob[:], fcob[:])
                nc.vector.tensor_scalar_add(yo[:], yo[:], fob[:])
                nc.sync.dma_start(out[:], yo[:])

    nc.compile()
    return nc


_CACHE: dict = {}


def _get_program(cfg: Cfg):
    key = cfg
    if key not in _CACHE:
        _CACHE[key] = build_program(cfg)
    return _CACHE[key]


def kernel(**inputs) -> np.ndarray:
    from concourse import bass_utils

    cfg = CFG
    nc = _get_program(cfg)
    in_maps = prep_core_inputs(cfg, **inputs)
    res = bass_utils.run_bass_kernel_spmd(
        nc, in_maps, core_ids=list(range(cfg.ncores)))
    return postprocess(res.results[0]["out"], cfg)
